# revision 47
# baseline (speedup 1.0000x reference)
"""Multi-head causal attention (B=4, T=2048, C=1024, H=16) on 8 trn2 cores.

Sharding: data-parallel over batch (4) x tensor-parallel over head-halves
(2, Megatron-style): core = 2*b + half computes heads [half*8, half*8+8)
for batch b.  Each core emits a PARTIAL transposed output
yT = Wo_slice.T @ attn_slice.T in [c_out, q] layout; the host sums the
two partials per batch, transposes, and adds bo + bv@Wo.T.

Key kernel tricks vs a naive port:
 - Q/K projections run in fp8 (e4m3, weights pre-scaled x64 out of the
   subnormal range; the exp scale divides the 64^2 back out) with
   DoubleRow perf mode: 256-deep contraction per instruction.
 - scores run with contraction 128 (full PE stream rate, not the h64
   half-rate): K-proj output is written "zebra"-style into two tiles per
   head pair, one head's 64 channels live, the other's rows zeroed, so
   the [128,128] stationary contracts 64 real + 64 zero rows.
 - attnV is transposed: stationary = v-block [128tok, 65], moving = expS
   [128tok, <=512 q], accumulating psoT[65, 2048] over key blocks with
   per-bank stop flags and early partial evictions.  v carries a ones
   column so psoT row 64 is the softmax denominator.
 - normalization: denom row -> DRAM -> re-read as [16,128] -> wide DVE
   reciprocal -> DRAM -> stride-0 broadcast DMA to 64 partitions -> one
   in-place multiply.  The reciprocal and multiply are emitted deferred,
   deep into the NEXT head's stream, so their DMA waits never block the
   DVE queue head (engine queues are in-order).
"""

import numpy as np
import ml_dtypes

import concourse.bass as bass
import concourse.mybir as mybir
import concourse.tile as tile
from concourse import bacc
from concourse.bass import ts
from concourse.bass_utils import run_bass_kernel_spmd

B, T, C, H, DK = 4, 2048, 1024, 16, 64
P = 128
NB = T // P            # 16 key/token blocks
HPC = H // 2           # 8 heads per core
KB = C // P            # 8 input-channel blocks
OB = HPC * DK // P     # 4 output-channel blocks (head pairs) per core
CPC = HPC * DK         # 512 channels per core
SCALE = 1.0 / np.sqrt(DK)
BF16 = mybir.dt.bfloat16
F32 = mybir.dt.float32
F8 = mybir.dt.float8e4
EXP = mybir.ActivationFunctionType.Exp
W8SCALE = 64.0   # fp8 weight pre-scale (keeps w out of subnormal range)

_cache = {}


def _build():
    nc = bacc.Bacc("TRN2", target_bir_lowering=False, debug=False)

    xT = nc.dram_tensor("xT", [C, T], BF16, kind="ExternalInput").ap()
    x8 = nc.dram_tensor("x8", [C, T], F8, kind="ExternalInput").ap()
    wq8 = nc.dram_tensor("wq8", [C, CPC], F8, kind="ExternalInput").ap()
    wk8 = nc.dram_tensor("wk8", [C, CPC], F8, kind="ExternalInput").ap()
    wv = nc.dram_tensor("wv", [C, CPC], BF16, kind="ExternalInput").ap()
    wo = nc.dram_tensor("wo", [CPC, C], BF16, kind="ExternalInput").ap()
    bq = nc.dram_tensor("bq", [P, OB], F32, kind="ExternalInput").ap()
    bkzA = nc.dram_tensor("bkzA", [P, OB], F32, kind="ExternalInput").ap()
    bkzB = nc.dram_tensor("bkzB", [P, OB], F32, kind="ExternalInput").ap()
    tri = nc.dram_tensor("tri", [P, P], BF16, kind="ExternalInput").ap()
    yT = nc.dram_tensor("yT", [C, T], BF16, kind="ExternalOutput").ap()

    with tile.TileContext(nc) as tc:
        with (
            tc.tile_pool(name="const", bufs=1) as cpool,
            tc.tile_pool(name="acc", bufs=1) as apool,
            tc.tile_pool(name="dram", bufs=1, space="DRAM") as dpool,
        ):
            tri_sb = cpool.tile([P, P], BF16)
            bq_sb = cpool.tile([P, OB], F32)
            bkzA_sb = cpool.tile([P, OB], F32)
            bkzB_sb = cpool.tile([P, OB], F32)
            z01 = cpool.tile([P, 1], F32)
            z10 = cpool.tile([P, 1], F32)
            wo_sb = cpool.tile([P, OB, C], BF16)
            nc.vector.memset(z01[0:DK, :], 1.0)
            nc.vector.memset(z01[DK:P, :], 0.0)
            nc.vector.memset(z10[0:DK, :], 0.0)
            nc.vector.memset(z10[DK:P, :], 1.0)

            aU = apool.tile([P, OB, T], BF16)     # attn.T, normalized in place
            recB = apool.tile([P, OB, T], BF16)   # 1/denom broadcast per head
            dscr = dpool.tile([HPC, T], F32)      # DRAM scratch: denom rows
            rscr = dpool.tile([HPC, T], BF16)     # DRAM scratch: recip rows

            with tc.tile_pool(name="qkv", bufs=1) as qkv:
                qT2 = qkv.tile([P, OB, T], BF16)        # q.T, dense head pairs
                kTzA = qkv.tile([P, OB, T], BF16)       # k.T, even head live
                kTzB = qkv.tile([P, OB, T], BF16)       # k.T, odd head live
                v_t = qkv.tile([P, NB, HPC, DK + 1], BF16)
                nc.vector.memset(v_t[:, :, :, DK : DK + 1], 1.0)

                # ---- phase 1: q/k/v projections ----
                with (
                    tc.tile_pool(name="xtw", bufs=1) as xtw,
                    tc.tile_pool(name="pacc", bufs=2, space="PSUM") as pacc,
                ):
                    xT_sb = xtw.tile([P, KB, T], BF16)
                    x8_sb = xtw.tile([P, KB, T], F8)
                    wq8_sb = xtw.tile([P, KB, CPC], F8)
                    wk8_sb = xtw.tile([P, KB, CPC], F8)
                    wv_sb = xtw.tile([P, KB, CPC], BF16)

                    wqr = wq8.rearrange("(kb p) o -> p kb o", p=P)
                    x8r = x8.rearrange("(kb p) t -> p kb t", p=P)
                    xTr = xT.rearrange("(kb p) t -> p kb t", p=P)
                    # kb-half granularity on the critical first loads so the
                    # first Q matmul starts as early as possible
                    nc.gpsimd.dma_start(wq8_sb[:, 0:4, :], wqr[:, 0:4, :])
                    nc.gpsimd.dma_start(x8_sb[:, 0:4, 0:512], x8r[:, 0:4, 0:512])
                    nc.gpsimd.dma_start(wq8_sb[:, 4:8, :], wqr[:, 4:8, :])
                    nc.gpsimd.dma_start(x8_sb[:, 4:8, 0:512], x8r[:, 4:8, 0:512])
                    nc.gpsimd.dma_start(x8_sb[:, :, 512:1024], x8r[:, :, 512:1024])
                    nc.gpsimd.dma_start(bq_sb[:], bq)
                    nc.gpsimd.dma_start(bkzA_sb[:], bkzA)
                    nc.gpsimd.dma_start(bkzB_sb[:], bkzB)
                    nc.gpsimd.dma_start(tri_sb[:], tri)
                    for tc4 in range(2, 4):
                        nc.gpsimd.dma_start(
                            x8_sb[:, :, ts(tc4, 512)], x8r[:, :, ts(tc4, 512)]
                        )
                    nc.gpsimd.dma_start(
                        wv_sb[:], wv.rearrange("(kb p) o -> p kb o", p=P)
                    )
                    for tc4 in range(4):
                        nc.gpsimd.dma_start(
                            xT_sb[:, :, ts(tc4, 512)], xTr[:, :, ts(tc4, 512)]
                        )
                    nc.gpsimd.dma_start(
                        wk8_sb[:], wk8.rearrange("(kb p) o -> p kb o", p=P)
                    )
                    nc.gpsimd.dma_start(
                        wo_sb[:], wo.rearrange("(cb p) o -> p cb o", p=P)
                    )

                    # Q projection -> qT2 [d(pair), pair, q]; fp8 DoubleRow
                    # contracts two 128-channel slabs per instruction
                    for tc4 in range(4):
                        accs = [
                            pacc.tile([P, 512], F32, tag=f"acc{ob}", name=f"qa{ob}")
                            for ob in range(OB)
                        ]
                        for kb2 in range(KB // 2):
                            for ob in range(OB):
                                nc.tensor.matmul(
                                    accs[ob][:],
                                    wq8_sb[:, 2 * kb2 : 2 * kb2 + 2, ts(ob, P)],
                                    x8_sb[:, 2 * kb2 : 2 * kb2 + 2, ts(tc4, 512)],
                                    start=(kb2 == 0),
                                    stop=(kb2 == KB // 2 - 1),
                                    perf_mode=mybir.MatmulPerfMode.DoubleRow,
                                )
                        for ob in range(OB):
                            nc.vector.tensor_scalar_add(
                                qT2[:, ob, ts(tc4, 512)],
                                accs[ob][:],
                                bq_sb[:, ob : ob + 1],
                            )

                    # V projection -> v_t [tok, tb, h, d] + ones column
                    for tb in range(NB):
                        acc = pacc.tile(
                            [P, 512], F32, tag=f"acc{tb % 4}", name="va"
                        )
                        for kb in range(KB):
                            nc.tensor.matmul(
                                acc[:],
                                xT_sb[:, kb, ts(tb, P)],
                                wv_sb[:, kb, :],
                                start=(kb == 0),
                                stop=(kb == KB - 1),
                            )
                        # eviction on ACT: DVE is the busier engine here
                        nc.scalar.copy(
                            v_t[:, tb, :, 0:DK],
                            acc[:].rearrange("p (h e) -> p h e", e=DK),
                        )

                    # K projection -> zebra tiles (last, so attention's ACT
                    # work can begin as soon as K lands)
                    for tc4 in range(4):
                        accs = [
                            pacc.tile([P, 512], F32, tag=f"acc{ob}", name=f"ka{ob}")
                            for ob in range(OB)
                        ]
                        for kb2 in range(KB // 2):
                            for ob in range(OB):
                                nc.tensor.matmul(
                                    accs[ob][:],
                                    wk8_sb[:, 2 * kb2 : 2 * kb2 + 2, ts(ob, P)],
                                    x8_sb[:, 2 * kb2 : 2 * kb2 + 2, ts(tc4, 512)],
                                    start=(kb2 == 0),
                                    stop=(kb2 == KB // 2 - 1),
                                    perf_mode=mybir.MatmulPerfMode.DoubleRow,
                                )
                        for ob in range(OB):
                            nc.vector.tensor_scalar(
                                kTzA[:, ob, ts(tc4, 512)],
                                accs[ob][:],
                                z01[:],
                                bkzA_sb[:, ob : ob + 1],
                                mybir.AluOpType.mult,
                                mybir.AluOpType.add,
                            )
                            if tc4 < 3:
                                # second zebra eviction on ACT (Identity with
                                # per-partition scale+bias) to halve DVE drain
                                nc.scalar.activation(
                                    kTzB[:, ob, ts(tc4, 512)],
                                    accs[ob][:],
                                    mybir.ActivationFunctionType.Identity,
                                    bias=bkzB_sb[:, ob : ob + 1],
                                    scale=z10[:],
                                )
                            else:
                                # last chunk on DVE so ACT is free to start
                                # the first exp immediately
                                nc.vector.tensor_scalar(
                                    kTzB[:, ob, ts(tc4, 512)],
                                    accs[ob][:],
                                    z10[:],
                                    bkzB_sb[:, ob : ob + 1],
                                    mybir.AluOpType.mult,
                                    mybir.AluOpType.add,
                                )

                # ---- phase 2: attention per head ----
                with (
                    tc.tile_pool(name="expp", bufs=3) as spool,
                    tc.tile_pool(name="recp", bufs=2) as rpool,
                    tc.tile_pool(name="ps_s", bufs=2, space="PSUM") as ps_s,
                    tc.tile_pool(name="ps_o", bufs=1, space="PSUM") as ps_o,
                ):
                    deferred_norm = [None]
                    deferred_mul = [None]
                    sched = []  # [jb, fn]: emitted when the head loop hits jb

                    def emit_norm_chain(h, pr, base, den1, lo, hi):
                        """Denominator -> reciprocal -> broadcast -> multiply
                        for q columns [lo, hi).  Returns (tail, mul) emitters
                        so callers can defer each past its DMA latency."""
                        nr = (hi - lo) // P
                        nc.gpsimd.dma_start(dscr[h : h + 1, lo:hi], den1[:, lo:hi])
                        denT = rpool.tile([NB, P], F32, tag="denT", name="denT")
                        nc.gpsimd.dma_start(
                            denT[0:nr, :],
                            dscr[h : h + 1, lo:hi].rearrange(
                                "o (p c) -> (o p) c", c=P
                            ),
                        )

                        def tail():
                            recT = rpool.tile([NB, P], BF16, tag="recT", name="recT")
                            with nc.allow_low_precision(reason="softmax recip"):
                                nc.vector.reciprocal(recT[0:nr, :], denT[0:nr, :])
                            nc.gpsimd.dma_start(
                                rscr[h : h + 1, lo:hi].rearrange(
                                    "o (p c) -> (o p) c", c=P
                                ),
                                recT[0:nr, :],
                            )
                            nc.gpsimd.dma_start(
                                recB[base : base + DK, pr, lo:hi],
                                rscr[h : h + 1, lo:hi].partition_broadcast(DK),
                            )

                        def mul():
                            nc.vector.tensor_mul(
                                aU[base : base + DK, pr, lo:hi],
                                aU[base : base + DK, pr, lo:hi],
                                recB[base : base + DK, pr, lo:hi],
                            )

                        return tail, mul

                    for h in range(HPC):
                        pr, hp = h // 2, h % 2
                        kTz = kTzA if hp == 0 else kTzB
                        base = hp * DK
                        psoT0 = ps_o.tile([P, 1024], F32, tag="pso0", name="psoT0")
                        psoT1 = ps_o.tile([P, 1024], F32, tag="pso1", name="psoT1")
                        den1 = rpool.tile([1, T], F32, tag="den1", name="den1")
                        pending = None

                        def emit_attnv(jb, expS_t):
                            q0 = P * jb
                            for c4 in range(q0 // 512, 4):
                                u = max(q0, 512 * c4)
                                w = 512 * (c4 + 1)
                                tgt, off = (psoT0, 0) if c4 < 2 else (psoT1, 1024)
                                nc.tensor.matmul(
                                    tgt[0:DK + 1, u - off:w - off],
                                    v_t[:, jb, h, :],
                                    expS_t[:, u:w],
                                    start=(jb == 0),
                                    stop=(jb == min(NB - 1, 4 * c4 + 3)),
                                    skip_group_check=True,
                                )
                            # half 0 of psoT is complete once jb 7 has landed:
                            # evict it early so the next head can reuse psum
                            if jb == 7:
                                nc.vector.tensor_copy(
                                    aU[base : base + DK, pr, 0:1024],
                                    psoT0[0:DK, :],
                                )
                                nc.vector.tensor_copy(
                                    den1[:, 0:1024], psoT0[DK : DK + 1, :]
                                )
                                if h >= HPC - 2:
                                    # last pair: normalize the first half now
                                    # so the output projection isn't gated on
                                    # the end-of-head chain
                                    t, m = emit_norm_chain(h, pr, base, den1, 0, 1024)
                                    t()
                                    sched.append([13, m])
                            # third quarter complete after jb 11: evicting it
                            # early shrinks the end-of-head psum release path
                            if jb == 11:
                                nc.vector.tensor_copy(
                                    aU[base : base + DK, pr, 1024:1536],
                                    psoT1[0:DK, 0:512],
                                )
                                nc.vector.tensor_copy(
                                    den1[:, 1024:1536], psoT1[DK : DK + 1, 0:512]
                                )
                                if h >= HPC - 2:
                                    t, m = emit_norm_chain(
                                        h, pr, base, den1, 1024, 1536
                                    )
                                    sched.append([14, t])
                                    sched.append([15, m])

                        for jb in range(NB):
                            q0 = P * jb
                            expS_t = spool.tile([P, T], BF16, tag="expS", name="eS")
                            for cc in range(q0 // 1024, 2):
                                lo = max(q0, 1024 * cc)
                                hi = 1024 * (cc + 1)
                                pss = ps_s.tile([P, 1024], F32, tag="pss", name="pss")
                                u = lo
                                while u < hi:
                                    w = min(hi, (u // 512 + 1) * 512)
                                    nc.tensor.matmul(
                                        pss[:, u - 1024 * cc : w - 1024 * cc],
                                        kTz[:, pr, ts(jb, P)],
                                        qT2[:, pr, u:w],
                                        start=True,
                                        stop=True,
                                    )
                                    u = w
                                nc.scalar.activation(
                                    expS_t[:, lo:hi],
                                    pss[:, lo - 1024 * cc : hi - 1024 * cc],
                                    EXP,
                                    scale=float(SCALE / (W8SCALE * W8SCALE)),
                                )
                            # causal mask on the diagonal block
                            nc.vector.tensor_mul(
                                expS_t[:, q0 : q0 + P],
                                expS_t[:, q0 : q0 + P],
                                tri_sb[:],
                            )
                            if jb == 4 and deferred_norm[0] is not None:
                                deferred_norm[0]()
                                deferred_norm[0] = None
                            if jb == 10 and deferred_mul[0] is not None:
                                deferred_mul[0]()
                                deferred_mul[0] = None
                            for e in list(sched):
                                if e[0] == jb:
                                    e[1]()
                                    sched.remove(e)
                            if pending is not None:
                                emit_attnv(*pending)
                            pending = (jb, expS_t)
                        emit_attnv(*pending)

                        # evict the last quarter; psoT is then free.  For the
                        # final head the evictions ride on ACT (its exp work
                        # is done) so the DVE queue stays clear for yproj.
                        if h == HPC - 1:
                            nc.scalar.copy(
                                aU[base : base + DK, pr, 1536:2048],
                                psoT1[0:DK, 512:1024],
                            )
                            nc.scalar.copy(
                                den1[:, 1536:2048], psoT1[DK : DK + 1, 512:1024]
                            )
                        else:
                            nc.vector.tensor_copy(
                                aU[base : base + DK, pr, 1536:2048],
                                psoT1[0:DK, 512:1024],
                            )
                            nc.vector.tensor_copy(
                                den1[:, 1536:2048], psoT1[DK : DK + 1, 512:1024]
                            )
                        if h == HPC - 2:
                            deferred_norm[0], deferred_mul[0] = emit_norm_chain(
                                h, pr, base, den1, 1536, 2048
                            )
                        elif h == HPC - 1:
                            t, m = emit_norm_chain(h, pr, base, den1, 1536, 2048)
                            t()
                            m()
                        else:
                            deferred_norm[0], deferred_mul[0] = emit_norm_chain(
                                h, pr, base, den1, 0, 2048
                            )
                    for dfr in (deferred_norm, deferred_mul):
                        if dfr[0] is not None:
                            dfr[0]()
                            dfr[0] = None
                    for e in sched:
                        e[1]()
                    sched.clear()

            # ---- phase 3: output projection (partial, transposed) ----
            with (
                tc.tile_pool(name="outp", bufs=6) as ypool,
                tc.tile_pool(name="ps_y", bufs=2, space="PSUM") as ps_y,
            ):
                yTr = yT.rearrange("(ob p) t -> p ob t", p=P)
                for qc4 in range(4):
                    for oblk in range(KB):
                        psy = ps_y.tile(
                            [P, 512], F32, tag=f"psy{oblk % 4}", name="psy"
                        )
                        for cb in range(OB):
                            nc.tensor.matmul(
                                psy[:],
                                wo_sb[:, cb, ts(oblk, P)],
                                aU[:, cb, ts(qc4, 512)],
                                start=(cb == 0),
                                stop=(cb == OB - 1),
                            )
                        ysb = ypool.tile([P, 512], BF16, tag="ysb", name="ysb")
                        if oblk % 2 == 0:
                            nc.vector.tensor_copy(ysb[:], psy[:])
                        else:
                            nc.scalar.copy(ysb[:], psy[:])
                        nc.gpsimd.dma_start(
                            yTr[:, oblk, ts(qc4, 512)], ysb[:]
                        )

    nc.compile()
    return nc


def _host_inputs(x, mask, Wq, bq_v, Wk, bk_v, Wv, bv_v, Wo, bo_v):
    f32 = np.float32
    bf16 = ml_dtypes.bfloat16
    f8 = ml_dtypes.float8_e4m3
    wqT = np.ascontiguousarray(np.asarray(Wq, f32).T) * W8SCALE
    wkT = np.ascontiguousarray(np.asarray(Wk, f32).T) * W8SCALE
    wvT = np.ascontiguousarray(np.asarray(Wv, f32).T).astype(bf16)
    woT = np.ascontiguousarray(np.asarray(Wo, f32).T).astype(bf16)
    # exact v/o bias fold: softmax rows sum to 1, so v+bv adds bv to attn out
    bo_eff = np.asarray(bo_v, f32) + np.asarray(bv_v, f32) @ np.asarray(Wo, f32).T

    # tri[k, q] = 1 where k <= q (causal keep within the diagonal block)
    kk = np.arange(P)
    tri = (kk[:, None] <= kk[None, :]).astype(f32).astype(bf16)

    bqa = np.asarray(bq_v, f32)
    bka = np.asarray(bk_v, f32)
    zebA = np.zeros((P, 1), f32)
    zebA[0:DK] = 1.0

    halves = []
    for half in range(2):
        sl = slice(half * CPC, (half + 1) * CPC)
        # biases carry the fp8 weight pre-scale; exp scale divides it out
        bq_p = np.ascontiguousarray(bqa[sl].reshape(OB, P).T) * W8SCALE
        bk_p = np.ascontiguousarray(bka[sl].reshape(OB, P).T) * W8SCALE
        halves.append(
            {
                "wq8": np.ascontiguousarray(wqT[:, sl]).astype(f8),
                "wk8": np.ascontiguousarray(wkT[:, sl]).astype(f8),
                "wv": np.ascontiguousarray(wvT[:, sl]),
                "wo": np.ascontiguousarray(woT[sl, :]),
                "bq": bq_p,
                "bkzA": bk_p * zebA,
                "bkzB": bk_p * (1.0 - zebA),
                "tri": tri,
            }
        )

    xn = np.asarray(x, f32)
    in_maps = []
    for core in range(8):
        b, half = divmod(core, 2)
        m = dict(halves[half])
        xTb = np.ascontiguousarray(xn[b].T)
        m["xT"] = xTb.astype(bf16)
        m["x8"] = xTb.astype(f8)
        in_maps.append(m)
    return in_maps, bo_eff


def _run(inputs, trace=False):
    if "nc" not in _cache:
        _cache["nc"] = _build()
    nc = _cache["nc"]
    in_maps, bo_eff = _host_inputs(
        inputs["x"], inputs["mask"],
        inputs["Wq"], inputs["bq"], inputs["Wk"], inputs["bk"],
        inputs["Wv"], inputs["bv"], inputs["Wo"], inputs["bo"],
    )
    res = run_bass_kernel_spmd(nc, in_maps, list(range(8)), trace=trace)
    out = np.empty((B, T, C), np.float32)
    for b in range(B):
        yA = res.results[2 * b]["yT"].astype(np.float32)
        yB = res.results[2 * b + 1]["yT"].astype(np.float32)
        out[b] = (yA + yB).T + bo_eff
    return out, res


def kernel(**inputs):
    out, _ = _run(inputs, trace=False)
    return out


# revision 48
# speedup vs baseline: 1.0189x; 1.0189x over previous
"""Multi-head causal attention (B=4, T=2048, C=1024, H=16) on 8 trn2 cores.

Sharding: data-parallel over batch (4) x tensor-parallel over head-halves
(2, Megatron-style): core = 2*b + half computes heads [half*8, half*8+8)
for batch b.  Each core emits a PARTIAL transposed output
yT = Wo_slice.T @ attn_slice.T in [c_out, q] layout; the host sums the
two partials per batch, transposes, and adds bo + bv@Wo.T.

Key kernel tricks vs a naive port:
 - scores run with contraction 128 (full PE stream rate, not the h64
   half-rate): K-proj output is written "zebra"-style into two tiles per
   head pair, one head's 64 channels live, the other's rows zeroed, so
   the [128,128] stationary contracts 64 real + 64 zero rows.
 - attnV is transposed: stationary = v-block [128tok, 65], moving = expS
   [128tok, <=512 q], accumulating psoT[65, 2048] over key blocks with
   per-bank stop flags.  v carries a ones column so psoT row 64 is the
   softmax denominator.
 - normalization: reciprocal of the denom row -> DRAM scratch ->
   stride-0 broadcast DMA back to 64 partitions -> one fused in-place
   multiply on the evicted attn tile.
"""

import numpy as np
import ml_dtypes

import concourse.bass as bass
import concourse.mybir as mybir
import concourse.tile as tile
from concourse import bacc
from concourse.bass import ts
from concourse.bass_utils import run_bass_kernel_spmd

B, T, C, H, DK = 4, 2048, 1024, 16, 64
P = 128
NB = T // P            # 16 key/token blocks
HPC = H // 2           # 8 heads per core
KB = C // P            # 8 input-channel blocks
OB = HPC * DK // P     # 4 output-channel blocks (head pairs) per core
CPC = HPC * DK         # 512 channels per core
SCALE = 1.0 / np.sqrt(DK)
BF16 = mybir.dt.bfloat16
F32 = mybir.dt.float32
F8 = mybir.dt.float8e4
EXP = mybir.ActivationFunctionType.Exp
W8SCALE = 64.0   # fp8 weight pre-scale (keeps w out of subnormal range)

_cache = {}


def _build():
    nc = bacc.Bacc("TRN2", target_bir_lowering=False, debug=False)

    xT = nc.dram_tensor("xT", [C, T], BF16, kind="ExternalInput").ap()
    x8 = nc.dram_tensor("x8", [C, T], F8, kind="ExternalInput").ap()
    wq8 = nc.dram_tensor("wq8", [C, CPC], F8, kind="ExternalInput").ap()
    wk8 = nc.dram_tensor("wk8", [C, CPC], F8, kind="ExternalInput").ap()
    wv = nc.dram_tensor("wv", [C, CPC], BF16, kind="ExternalInput").ap()
    wo = nc.dram_tensor("wo", [CPC, C], BF16, kind="ExternalInput").ap()
    bq = nc.dram_tensor("bq", [P, OB], F32, kind="ExternalInput").ap()
    bkzA = nc.dram_tensor("bkzA", [P, OB], F32, kind="ExternalInput").ap()
    bkzB = nc.dram_tensor("bkzB", [P, OB], F32, kind="ExternalInput").ap()
    tri = nc.dram_tensor("tri", [P, P], BF16, kind="ExternalInput").ap()
    yT = nc.dram_tensor("yT", [C, T], BF16, kind="ExternalOutput").ap()

    with tile.TileContext(nc) as tc:
        with (
            tc.tile_pool(name="const", bufs=1) as cpool,
            tc.tile_pool(name="acc", bufs=1) as apool,
            tc.tile_pool(name="dram", bufs=1, space="DRAM") as dpool,
        ):
            tri_sb = cpool.tile([P, P], BF16)
            bq_sb = cpool.tile([P, OB], F32)
            bkzA_sb = cpool.tile([P, OB], F32)
            bkzB_sb = cpool.tile([P, OB], F32)
            z01 = cpool.tile([P, 1], F32)
            z10 = cpool.tile([P, 1], F32)
            wo_sb = cpool.tile([P, OB, C], BF16)
            nc.vector.memset(z01[0:DK, :], 1.0)
            nc.vector.memset(z01[DK:P, :], 0.0)
            nc.vector.memset(z10[0:DK, :], 0.0)
            nc.vector.memset(z10[DK:P, :], 1.0)

            aU = apool.tile([P, OB, T], BF16)     # attn.T, normalized in place
            recB = apool.tile([P, OB, T], BF16)   # 1/denom broadcast per head
            dscr = dpool.tile([HPC, T], F32)      # DRAM scratch: denom rows
            rscr = dpool.tile([HPC, T], BF16)     # DRAM scratch: recip rows

            with tc.tile_pool(name="qkv", bufs=1) as qkv:
                qT2 = qkv.tile([P, OB, T], BF16)        # q.T, dense head pairs
                kTzA = qkv.tile([P, OB, T], BF16)       # k.T, even head live
                kTzB = qkv.tile([P, OB, T], BF16)       # k.T, odd head live
                v_t = qkv.tile([P, NB, HPC, DK + 1], BF16)
                nc.vector.memset(v_t[:, :, :, DK : DK + 1], 1.0)

                # ---- phase 1: q/k/v projections ----
                with (
                    tc.tile_pool(name="xtw", bufs=1) as xtw,
                    tc.tile_pool(name="pacc", bufs=2, space="PSUM") as pacc,
                ):
                    xT_sb = xtw.tile([P, KB, T], BF16)
                    x8_sb = xtw.tile([P, KB, T], F8)
                    wq8_sb = xtw.tile([P, KB, CPC], F8)
                    wk8_sb = xtw.tile([P, KB, CPC], F8)
                    wv_sb = xtw.tile([P, KB, CPC], BF16)

                    wqr = wq8.rearrange("(kb p) o -> p kb o", p=P)
                    x8r = x8.rearrange("(kb p) t -> p kb t", p=P)
                    xTr = xT.rearrange("(kb p) t -> p kb t", p=P)
                    # kb-half granularity on the critical first loads so the
                    # first Q matmul starts as early as possible
                    nc.gpsimd.dma_start(wq8_sb[:, 0:4, :], wqr[:, 0:4, :])
                    nc.gpsimd.dma_start(x8_sb[:, 0:4, 0:512], x8r[:, 0:4, 0:512])
                    nc.gpsimd.dma_start(wq8_sb[:, 4:8, :], wqr[:, 4:8, :])
                    nc.gpsimd.dma_start(x8_sb[:, 4:8, 0:512], x8r[:, 4:8, 0:512])
                    nc.gpsimd.dma_start(x8_sb[:, :, 512:1024], x8r[:, :, 512:1024])
                    nc.gpsimd.dma_start(bq_sb[:], bq)
                    nc.gpsimd.dma_start(bkzA_sb[:], bkzA)
                    nc.gpsimd.dma_start(bkzB_sb[:], bkzB)
                    nc.gpsimd.dma_start(tri_sb[:], tri)
                    for tc4 in range(2, 4):
                        nc.gpsimd.dma_start(
                            x8_sb[:, :, ts(tc4, 512)], x8r[:, :, ts(tc4, 512)]
                        )
                    nc.gpsimd.dma_start(
                        wv_sb[:], wv.rearrange("(kb p) o -> p kb o", p=P)
                    )
                    for tc4 in range(4):
                        nc.gpsimd.dma_start(
                            xT_sb[:, :, ts(tc4, 512)], xTr[:, :, ts(tc4, 512)]
                        )
                    nc.gpsimd.dma_start(
                        wk8_sb[:], wk8.rearrange("(kb p) o -> p kb o", p=P)
                    )
                    nc.gpsimd.dma_start(
                        wo_sb[:], wo.rearrange("(cb p) o -> p cb o", p=P)
                    )

                    # Q projection -> qT2 [d(pair), pair, q]; fp8 DoubleRow
                    # contracts two 128-channel slabs per instruction
                    for tc4 in range(4):
                        accs = [
                            pacc.tile([P, 512], F32, tag=f"acc{ob}", name=f"qa{ob}")
                            for ob in range(OB)
                        ]
                        for kb2 in range(KB // 2):
                            for ob in range(OB):
                                nc.tensor.matmul(
                                    accs[ob][:],
                                    wq8_sb[:, 2 * kb2 : 2 * kb2 + 2, ts(ob, P)],
                                    x8_sb[:, 2 * kb2 : 2 * kb2 + 2, ts(tc4, 512)],
                                    start=(kb2 == 0),
                                    stop=(kb2 == KB // 2 - 1),
                                    perf_mode=mybir.MatmulPerfMode.DoubleRow,
                                )
                        for ob in range(OB):
                            nc.vector.tensor_scalar_add(
                                qT2[:, ob, ts(tc4, 512)],
                                accs[ob][:],
                                bq_sb[:, ob : ob + 1],
                            )

                    # V projection -> v_t [tok, tb, h, d] + ones column
                    for tb in range(NB):
                        acc = pacc.tile(
                            [P, 512], F32, tag=f"acc{tb % 4}", name="va"
                        )
                        for kb in range(KB):
                            nc.tensor.matmul(
                                acc[:],
                                xT_sb[:, kb, ts(tb, P)],
                                wv_sb[:, kb, :],
                                start=(kb == 0),
                                stop=(kb == KB - 1),
                            )
                        # eviction on ACT: DVE is the busier engine here
                        nc.scalar.copy(
                            v_t[:, tb, :, 0:DK],
                            acc[:].rearrange("p (h e) -> p h e", e=DK),
                        )

                    # K projection -> zebra tiles (last, so attention's ACT
                    # work can begin as soon as K lands)
                    for tc4 in range(4):
                        accs = [
                            pacc.tile([P, 512], F32, tag=f"acc{ob}", name=f"ka{ob}")
                            for ob in range(OB)
                        ]
                        for kb2 in range(KB // 2):
                            for ob in range(OB):
                                nc.tensor.matmul(
                                    accs[ob][:],
                                    wk8_sb[:, 2 * kb2 : 2 * kb2 + 2, ts(ob, P)],
                                    x8_sb[:, 2 * kb2 : 2 * kb2 + 2, ts(tc4, 512)],
                                    start=(kb2 == 0),
                                    stop=(kb2 == KB // 2 - 1),
                                    perf_mode=mybir.MatmulPerfMode.DoubleRow,
                                )
                        for ob in range(OB):
                            nc.vector.tensor_scalar(
                                kTzA[:, ob, ts(tc4, 512)],
                                accs[ob][:],
                                z01[:],
                                bkzA_sb[:, ob : ob + 1],
                                mybir.AluOpType.mult,
                                mybir.AluOpType.add,
                            )
                            # second zebra eviction on ACT (Identity with
                            # per-partition scale+bias) to halve DVE drain
                            nc.scalar.activation(
                                kTzB[:, ob, ts(tc4, 512)],
                                accs[ob][:],
                                mybir.ActivationFunctionType.Identity,
                                bias=bkzB_sb[:, ob : ob + 1],
                                scale=z10[:],
                            )

                # ---- phase 2: attention per head ----
                with (
                    tc.tile_pool(name="expp", bufs=3) as spool,
                    tc.tile_pool(name="recp", bufs=2) as rpool,
                    tc.tile_pool(name="ps_s", bufs=2, space="PSUM") as ps_s,
                    tc.tile_pool(name="ps_o", bufs=1, space="PSUM") as ps_o,
                ):
                    deferred_norm = [None]
                    deferred_mul = [None]
                    self_mul = [None]

                    def emit_norm_chain(h, pr, base, den1, lo, hi):
                        """Denominator -> reciprocal -> broadcast -> multiply
                        for q columns [lo, hi).  Returns (tail, mul) emitters
                        so callers can defer each past its DMA latency."""
                        nr = (hi - lo) // P
                        nc.gpsimd.dma_start(dscr[h : h + 1, lo:hi], den1[:, lo:hi])
                        denT = rpool.tile([NB, P], F32, tag="denT", name="denT")
                        nc.gpsimd.dma_start(
                            denT[0:nr, :],
                            dscr[h : h + 1, lo:hi].rearrange(
                                "o (p c) -> (o p) c", c=P
                            ),
                        )

                        def tail():
                            recT = rpool.tile([NB, P], BF16, tag="recT", name="recT")
                            with nc.allow_low_precision(reason="softmax recip"):
                                nc.vector.reciprocal(recT[0:nr, :], denT[0:nr, :])
                            nc.gpsimd.dma_start(
                                rscr[h : h + 1, lo:hi].rearrange(
                                    "o (p c) -> (o p) c", c=P
                                ),
                                recT[0:nr, :],
                            )
                            nc.gpsimd.dma_start(
                                recB[base : base + DK, pr, lo:hi],
                                rscr[h : h + 1, lo:hi].partition_broadcast(DK),
                            )

                        def mul():
                            nc.vector.tensor_mul(
                                aU[base : base + DK, pr, lo:hi],
                                aU[base : base + DK, pr, lo:hi],
                                recB[base : base + DK, pr, lo:hi],
                            )

                        return tail, mul

                    for h in range(HPC):
                        pr, hp = h // 2, h % 2
                        kTz = kTzA if hp == 0 else kTzB
                        base = hp * DK
                        psoT0 = ps_o.tile([P, 1024], F32, tag="pso0", name="psoT0")
                        psoT1 = ps_o.tile([P, 1024], F32, tag="pso1", name="psoT1")
                        den1 = rpool.tile([1, T], F32, tag="den1", name="den1")
                        pending = None

                        def emit_attnv(jb, expS_t):
                            q0 = P * jb
                            for c4 in range(q0 // 512, 4):
                                u = max(q0, 512 * c4)
                                w = 512 * (c4 + 1)
                                tgt, off = (psoT0, 0) if c4 < 2 else (psoT1, 1024)
                                nc.tensor.matmul(
                                    tgt[0:DK + 1, u - off:w - off],
                                    v_t[:, jb, h, :],
                                    expS_t[:, u:w],
                                    start=(jb == 0),
                                    stop=(jb == min(NB - 1, 4 * c4 + 3)),
                                    skip_group_check=True,
                                )
                            # half 0 of psoT is complete once jb 7 has landed:
                            # evict it early so the next head can reuse psum
                            if jb == 7:
                                nc.vector.tensor_copy(
                                    aU[base : base + DK, pr, 0:1024],
                                    psoT0[0:DK, :],
                                )
                                nc.vector.tensor_copy(
                                    den1[:, 0:1024], psoT0[DK : DK + 1, :]
                                )
                                if h >= HPC - 2:
                                    # last pair: normalize the first half now
                                    # so the output projection isn't gated on
                                    # the end-of-head chain
                                    t, m = emit_norm_chain(h, pr, base, den1, 0, 1024)
                                    t()
                                    self_mul[0] = m
                            # third quarter complete after jb 11: evicting it
                            # early shrinks the end-of-head psum release path
                            if jb == 11:
                                nc.vector.tensor_copy(
                                    aU[base : base + DK, pr, 1024:1536],
                                    psoT1[0:DK, 0:512],
                                )
                                nc.vector.tensor_copy(
                                    den1[:, 1024:1536], psoT1[DK : DK + 1, 0:512]
                                )

                        for jb in range(NB):
                            q0 = P * jb
                            expS_t = spool.tile([P, T], BF16, tag="expS", name="eS")
                            for cc in range(q0 // 1024, 2):
                                lo = max(q0, 1024 * cc)
                                hi = 1024 * (cc + 1)
                                pss = ps_s.tile([P, 1024], F32, tag="pss", name="pss")
                                u = lo
                                while u < hi:
                                    w = min(hi, (u // 512 + 1) * 512)
                                    nc.tensor.matmul(
                                        pss[:, u - 1024 * cc : w - 1024 * cc],
                                        kTz[:, pr, ts(jb, P)],
                                        qT2[:, pr, u:w],
                                        start=True,
                                        stop=True,
                                    )
                                    u = w
                                nc.scalar.activation(
                                    expS_t[:, lo:hi],
                                    pss[:, lo - 1024 * cc : hi - 1024 * cc],
                                    EXP,
                                    scale=float(SCALE / (W8SCALE * W8SCALE)),
                                )
                            # causal mask on the diagonal block
                            nc.vector.tensor_mul(
                                expS_t[:, q0 : q0 + P],
                                expS_t[:, q0 : q0 + P],
                                tri_sb[:],
                            )
                            if jb == 4 and deferred_norm[0] is not None:
                                deferred_norm[0]()
                                deferred_norm[0] = None
                            if jb == 10 and deferred_mul[0] is not None:
                                deferred_mul[0]()
                                deferred_mul[0] = None
                            if jb == 13 and self_mul[0] is not None:
                                self_mul[0]()
                                self_mul[0] = None
                            if pending is not None:
                                emit_attnv(*pending)
                            pending = (jb, expS_t)
                        emit_attnv(*pending)

                        # evict the last quarter; psoT is then free
                        nc.vector.tensor_copy(
                            aU[base : base + DK, pr, 1536:2048],
                            psoT1[0:DK, 512:1024],
                        )
                        nc.vector.tensor_copy(
                            den1[:, 1536:2048], psoT1[DK : DK + 1, 512:1024]
                        )
                        if h == HPC - 2:
                            deferred_norm[0], deferred_mul[0] = emit_norm_chain(
                                h, pr, base, den1, 1024, 2048
                            )
                        elif h == HPC - 1:
                            t, m = emit_norm_chain(h, pr, base, den1, 1024, 2048)
                            t()
                            m()
                        else:
                            deferred_norm[0], deferred_mul[0] = emit_norm_chain(
                                h, pr, base, den1, 0, 2048
                            )
                    for dfr in (deferred_norm, deferred_mul, self_mul):
                        if dfr[0] is not None:
                            dfr[0]()
                            dfr[0] = None

            # ---- phase 3: output projection (partial, transposed) ----
            with (
                tc.tile_pool(name="outp", bufs=6) as ypool,
                tc.tile_pool(name="ps_y", bufs=2, space="PSUM") as ps_y,
            ):
                yTr = yT.rearrange("(ob p) t -> p ob t", p=P)
                for qc4 in range(4):
                    for oblk in range(KB):
                        psy = ps_y.tile(
                            [P, 512], F32, tag=f"psy{oblk % 4}", name="psy"
                        )
                        for cb in range(OB):
                            nc.tensor.matmul(
                                psy[:],
                                wo_sb[:, cb, ts(oblk, P)],
                                aU[:, cb, ts(qc4, 512)],
                                start=(cb == 0),
                                stop=(cb == OB - 1),
                            )
                        ysb = ypool.tile([P, 512], BF16, tag="ysb", name="ysb")
                        if oblk % 2 == 0:
                            nc.vector.tensor_copy(ysb[:], psy[:])
                        else:
                            nc.scalar.copy(ysb[:], psy[:])
                        nc.gpsimd.dma_start(
                            yTr[:, oblk, ts(qc4, 512)], ysb[:]
                        )

    nc.compile()
    return nc


def _host_inputs(x, mask, Wq, bq_v, Wk, bk_v, Wv, bv_v, Wo, bo_v):
    f32 = np.float32
    bf16 = ml_dtypes.bfloat16
    f8 = ml_dtypes.float8_e4m3
    wqT = np.ascontiguousarray(np.asarray(Wq, f32).T) * W8SCALE
    wkT = np.ascontiguousarray(np.asarray(Wk, f32).T) * W8SCALE
    wvT = np.ascontiguousarray(np.asarray(Wv, f32).T).astype(bf16)
    woT = np.ascontiguousarray(np.asarray(Wo, f32).T).astype(bf16)
    # exact v/o bias fold: softmax rows sum to 1, so v+bv adds bv to attn out
    bo_eff = np.asarray(bo_v, f32) + np.asarray(bv_v, f32) @ np.asarray(Wo, f32).T

    # tri[k, q] = 1 where k <= q (causal keep within the diagonal block)
    kk = np.arange(P)
    tri = (kk[:, None] <= kk[None, :]).astype(f32).astype(bf16)

    bqa = np.asarray(bq_v, f32)
    bka = np.asarray(bk_v, f32)
    zebA = np.zeros((P, 1), f32)
    zebA[0:DK] = 1.0

    halves = []
    for half in range(2):
        sl = slice(half * CPC, (half + 1) * CPC)
        # biases carry the fp8 weight pre-scale; exp scale divides it out
        bq_p = np.ascontiguousarray(bqa[sl].reshape(OB, P).T) * W8SCALE
        bk_p = np.ascontiguousarray(bka[sl].reshape(OB, P).T) * W8SCALE
        halves.append(
            {
                "wq8": np.ascontiguousarray(wqT[:, sl]).astype(f8),
                "wk8": np.ascontiguousarray(wkT[:, sl]).astype(f8),
                "wv": np.ascontiguousarray(wvT[:, sl]),
                "wo": np.ascontiguousarray(woT[sl, :]),
                "bq": bq_p,
                "bkzA": bk_p * zebA,
                "bkzB": bk_p * (1.0 - zebA),
                "tri": tri,
            }
        )

    xn = np.asarray(x, f32)
    in_maps = []
    for core in range(8):
        b, half = divmod(core, 2)
        m = dict(halves[half])
        xTb = np.ascontiguousarray(xn[b].T)
        m["xT"] = xTb.astype(bf16)
        m["x8"] = xTb.astype(f8)
        in_maps.append(m)
    return in_maps, bo_eff


def _run(inputs, trace=False):
    if "nc" not in _cache:
        _cache["nc"] = _build()
    nc = _cache["nc"]
    in_maps, bo_eff = _host_inputs(
        inputs["x"], inputs["mask"],
        inputs["Wq"], inputs["bq"], inputs["Wk"], inputs["bk"],
        inputs["Wv"], inputs["bv"], inputs["Wo"], inputs["bo"],
    )
    res = run_bass_kernel_spmd(nc, in_maps, list(range(8)), trace=trace)
    out = np.empty((B, T, C), np.float32)
    for b in range(B):
        yA = res.results[2 * b]["yT"].astype(np.float32)
        yB = res.results[2 * b + 1]["yT"].astype(np.float32)
        out[b] = (yA + yB).T + bo_eff
    return out, res


def kernel(**inputs):
    out, _ = _run(inputs, trace=False)
    return out


# revision 49
# speedup vs baseline: 1.0228x; 1.0039x over previous
"""Multi-head causal attention (B=4, T=2048, C=1024, H=16) on 8 trn2 cores.

Sharding: data-parallel over batch (4) x tensor-parallel over head-halves
(2, Megatron-style): core = 2*b + half computes heads [half*8, half*8+8)
for batch b.  Each core emits a PARTIAL transposed output
yT = Wo_slice.T @ attn_slice.T in [c_out, q] layout; the host sums the
two partials per batch, transposes, and adds bo + bv@Wo.T.

Key kernel tricks vs a naive port:
 - scores run with contraction 128 (full PE stream rate, not the h64
   half-rate): K-proj output is written "zebra"-style into two tiles per
   head pair, one head's 64 channels live, the other's rows zeroed, so
   the [128,128] stationary contracts 64 real + 64 zero rows.
 - attnV is transposed: stationary = v-block [128tok, 65], moving = expS
   [128tok, <=512 q], accumulating psoT[65, 2048] over key blocks with
   per-bank stop flags.  v carries a ones column so psoT row 64 is the
   softmax denominator.
 - normalization: reciprocal of the denom row -> DRAM scratch ->
   stride-0 broadcast DMA back to 64 partitions -> one fused in-place
   multiply on the evicted attn tile.
"""

import numpy as np
import ml_dtypes

import concourse.bass as bass
import concourse.mybir as mybir
import concourse.tile as tile
from concourse import bacc
from concourse.bass import ts
from concourse.bass_utils import run_bass_kernel_spmd

B, T, C, H, DK = 4, 2048, 1024, 16, 64
P = 128
NB = T // P            # 16 key/token blocks
HPC = H // 2           # 8 heads per core
KB = C // P            # 8 input-channel blocks
OB = HPC * DK // P     # 4 output-channel blocks (head pairs) per core
CPC = HPC * DK         # 512 channels per core
SCALE = 1.0 / np.sqrt(DK)
BF16 = mybir.dt.bfloat16
F32 = mybir.dt.float32
F8 = mybir.dt.float8e4
EXP = mybir.ActivationFunctionType.Exp
W8SCALE = 64.0   # fp8 weight pre-scale (keeps w out of subnormal range)

_cache = {}


def _build():
    nc = bacc.Bacc("TRN2", target_bir_lowering=False, debug=False)

    xT = nc.dram_tensor("xT", [C, T], BF16, kind="ExternalInput").ap()
    x8 = nc.dram_tensor("x8", [C, T], F8, kind="ExternalInput").ap()
    wq8 = nc.dram_tensor("wq8", [C, CPC], F8, kind="ExternalInput").ap()
    wk8 = nc.dram_tensor("wk8", [C, CPC], F8, kind="ExternalInput").ap()
    wv = nc.dram_tensor("wv", [C, CPC], BF16, kind="ExternalInput").ap()
    wo = nc.dram_tensor("wo", [CPC, C], BF16, kind="ExternalInput").ap()
    bq = nc.dram_tensor("bq", [P, OB], F32, kind="ExternalInput").ap()
    bkzA = nc.dram_tensor("bkzA", [P, OB], F32, kind="ExternalInput").ap()
    bkzB = nc.dram_tensor("bkzB", [P, OB], F32, kind="ExternalInput").ap()
    tri = nc.dram_tensor("tri", [P, P], BF16, kind="ExternalInput").ap()
    yT = nc.dram_tensor("yT", [C, T], BF16, kind="ExternalOutput").ap()

    with tile.TileContext(nc) as tc:
        with (
            tc.tile_pool(name="const", bufs=1) as cpool,
            tc.tile_pool(name="acc", bufs=1) as apool,
            tc.tile_pool(name="dram", bufs=1, space="DRAM") as dpool,
        ):
            tri_sb = cpool.tile([P, P], BF16)
            bq_sb = cpool.tile([P, OB], F32)
            bkzA_sb = cpool.tile([P, OB], F32)
            bkzB_sb = cpool.tile([P, OB], F32)
            z01 = cpool.tile([P, 1], F32)
            z10 = cpool.tile([P, 1], F32)
            wo_sb = cpool.tile([P, OB, C], BF16)
            nc.vector.memset(z01[0:DK, :], 1.0)
            nc.vector.memset(z01[DK:P, :], 0.0)
            nc.vector.memset(z10[0:DK, :], 0.0)
            nc.vector.memset(z10[DK:P, :], 1.0)

            aU = apool.tile([P, OB, T], BF16)     # attn.T, normalized in place
            recB = apool.tile([P, OB, T], BF16)   # 1/denom broadcast per head
            dscr = dpool.tile([HPC, T], F32)      # DRAM scratch: denom rows
            rscr = dpool.tile([HPC, T], BF16)     # DRAM scratch: recip rows

            with tc.tile_pool(name="qkv", bufs=1) as qkv:
                qT2 = qkv.tile([P, OB, T], BF16)        # q.T, dense head pairs
                kTzA = qkv.tile([P, OB, T], BF16)       # k.T, even head live
                kTzB = qkv.tile([P, OB, T], BF16)       # k.T, odd head live
                v_t = qkv.tile([P, NB, HPC, DK + 1], BF16)
                nc.vector.memset(v_t[:, :, :, DK : DK + 1], 1.0)

                # ---- phase 1: q/k/v projections ----
                with (
                    tc.tile_pool(name="xtw", bufs=1) as xtw,
                    tc.tile_pool(name="pacc", bufs=2, space="PSUM") as pacc,
                ):
                    xT_sb = xtw.tile([P, KB, T], BF16)
                    x8_sb = xtw.tile([P, KB, T], F8)
                    wq8_sb = xtw.tile([P, KB, CPC], F8)
                    wk8_sb = xtw.tile([P, KB, CPC], F8)
                    wv_sb = xtw.tile([P, KB, CPC], BF16)

                    wqr = wq8.rearrange("(kb p) o -> p kb o", p=P)
                    x8r = x8.rearrange("(kb p) t -> p kb t", p=P)
                    xTr = xT.rearrange("(kb p) t -> p kb t", p=P)
                    # kb-half granularity on the critical first loads so the
                    # first Q matmul starts as early as possible
                    nc.gpsimd.dma_start(wq8_sb[:, 0:4, :], wqr[:, 0:4, :])
                    nc.gpsimd.dma_start(x8_sb[:, 0:4, 0:512], x8r[:, 0:4, 0:512])
                    nc.gpsimd.dma_start(wq8_sb[:, 4:8, :], wqr[:, 4:8, :])
                    nc.gpsimd.dma_start(x8_sb[:, 4:8, 0:512], x8r[:, 4:8, 0:512])
                    nc.gpsimd.dma_start(x8_sb[:, :, 512:1024], x8r[:, :, 512:1024])
                    nc.gpsimd.dma_start(bq_sb[:], bq)
                    nc.gpsimd.dma_start(bkzA_sb[:], bkzA)
                    nc.gpsimd.dma_start(bkzB_sb[:], bkzB)
                    nc.gpsimd.dma_start(tri_sb[:], tri)
                    for tc4 in range(2, 4):
                        nc.gpsimd.dma_start(
                            x8_sb[:, :, ts(tc4, 512)], x8r[:, :, ts(tc4, 512)]
                        )
                    nc.gpsimd.dma_start(
                        wv_sb[:], wv.rearrange("(kb p) o -> p kb o", p=P)
                    )
                    for tc4 in range(4):
                        nc.gpsimd.dma_start(
                            xT_sb[:, :, ts(tc4, 512)], xTr[:, :, ts(tc4, 512)]
                        )
                    nc.gpsimd.dma_start(
                        wk8_sb[:], wk8.rearrange("(kb p) o -> p kb o", p=P)
                    )
                    nc.gpsimd.dma_start(
                        wo_sb[:], wo.rearrange("(cb p) o -> p cb o", p=P)
                    )

                    # Q projection -> qT2 [d(pair), pair, q]; fp8 DoubleRow
                    # contracts two 128-channel slabs per instruction
                    for tc4 in range(4):
                        accs = [
                            pacc.tile([P, 512], F32, tag=f"acc{ob}", name=f"qa{ob}")
                            for ob in range(OB)
                        ]
                        for kb2 in range(KB // 2):
                            for ob in range(OB):
                                nc.tensor.matmul(
                                    accs[ob][:],
                                    wq8_sb[:, 2 * kb2 : 2 * kb2 + 2, ts(ob, P)],
                                    x8_sb[:, 2 * kb2 : 2 * kb2 + 2, ts(tc4, 512)],
                                    start=(kb2 == 0),
                                    stop=(kb2 == KB // 2 - 1),
                                    perf_mode=mybir.MatmulPerfMode.DoubleRow,
                                )
                        for ob in range(OB):
                            nc.vector.tensor_scalar_add(
                                qT2[:, ob, ts(tc4, 512)],
                                accs[ob][:],
                                bq_sb[:, ob : ob + 1],
                            )

                    # V projection -> v_t [tok, tb, h, d] + ones column
                    for tb in range(NB):
                        acc = pacc.tile(
                            [P, 512], F32, tag=f"acc{tb % 4}", name="va"
                        )
                        for kb in range(KB):
                            nc.tensor.matmul(
                                acc[:],
                                xT_sb[:, kb, ts(tb, P)],
                                wv_sb[:, kb, :],
                                start=(kb == 0),
                                stop=(kb == KB - 1),
                            )
                        # eviction on ACT: DVE is the busier engine here
                        nc.scalar.copy(
                            v_t[:, tb, :, 0:DK],
                            acc[:].rearrange("p (h e) -> p h e", e=DK),
                        )

                    # K projection -> zebra tiles (last, so attention's ACT
                    # work can begin as soon as K lands)
                    for tc4 in range(4):
                        accs = [
                            pacc.tile([P, 512], F32, tag=f"acc{ob}", name=f"ka{ob}")
                            for ob in range(OB)
                        ]
                        for kb2 in range(KB // 2):
                            for ob in range(OB):
                                nc.tensor.matmul(
                                    accs[ob][:],
                                    wk8_sb[:, 2 * kb2 : 2 * kb2 + 2, ts(ob, P)],
                                    x8_sb[:, 2 * kb2 : 2 * kb2 + 2, ts(tc4, 512)],
                                    start=(kb2 == 0),
                                    stop=(kb2 == KB // 2 - 1),
                                    perf_mode=mybir.MatmulPerfMode.DoubleRow,
                                )
                        for ob in range(OB):
                            nc.vector.tensor_scalar(
                                kTzA[:, ob, ts(tc4, 512)],
                                accs[ob][:],
                                z01[:],
                                bkzA_sb[:, ob : ob + 1],
                                mybir.AluOpType.mult,
                                mybir.AluOpType.add,
                            )
                            # second zebra eviction on ACT (Identity with
                            # per-partition scale+bias) to halve DVE drain
                            nc.scalar.activation(
                                kTzB[:, ob, ts(tc4, 512)],
                                accs[ob][:],
                                mybir.ActivationFunctionType.Identity,
                                bias=bkzB_sb[:, ob : ob + 1],
                                scale=z10[:],
                            )

                # ---- phase 2: attention per head ----
                with (
                    tc.tile_pool(name="expp", bufs=3) as spool,
                    tc.tile_pool(name="recp", bufs=2) as rpool,
                    tc.tile_pool(name="ps_s", bufs=2, space="PSUM") as ps_s,
                    tc.tile_pool(name="ps_o", bufs=1, space="PSUM") as ps_o,
                ):
                    deferred_norm = [None]
                    deferred_mul = [None]
                    self_mul = [None]

                    def emit_norm_chain(h, pr, base, den1, lo, hi):
                        """Denominator -> reciprocal -> broadcast -> multiply
                        for q columns [lo, hi).  Returns (tail, mul) emitters
                        so callers can defer each past its DMA latency."""
                        nr = (hi - lo) // P
                        nc.sync.dma_start(dscr[h : h + 1, lo:hi], den1[:, lo:hi])
                        denT = rpool.tile([NB, P], F32, tag="denT", name="denT")
                        nc.sync.dma_start(
                            denT[0:nr, :],
                            dscr[h : h + 1, lo:hi].rearrange(
                                "o (p c) -> (o p) c", c=P
                            ),
                        )

                        def tail():
                            recT = rpool.tile([NB, P], BF16, tag="recT", name="recT")
                            with nc.allow_low_precision(reason="softmax recip"):
                                nc.vector.reciprocal(recT[0:nr, :], denT[0:nr, :])
                            nc.sync.dma_start(
                                rscr[h : h + 1, lo:hi].rearrange(
                                    "o (p c) -> (o p) c", c=P
                                ),
                                recT[0:nr, :],
                            )
                            nc.gpsimd.dma_start(
                                recB[base : base + DK, pr, lo:hi],
                                rscr[h : h + 1, lo:hi].partition_broadcast(DK),
                            )

                        def mul():
                            nc.vector.tensor_mul(
                                aU[base : base + DK, pr, lo:hi],
                                aU[base : base + DK, pr, lo:hi],
                                recB[base : base + DK, pr, lo:hi],
                            )

                        return tail, mul

                    for h in range(HPC):
                        pr, hp = h // 2, h % 2
                        kTz = kTzA if hp == 0 else kTzB
                        base = hp * DK
                        psoT0 = ps_o.tile([P, 1024], F32, tag="pso0", name="psoT0")
                        psoT1 = ps_o.tile([P, 1024], F32, tag="pso1", name="psoT1")
                        den1 = rpool.tile([1, T], F32, tag="den1", name="den1")
                        pending = None

                        def emit_attnv(jb, expS_t):
                            q0 = P * jb
                            for c4 in range(q0 // 512, 4):
                                u = max(q0, 512 * c4)
                                w = 512 * (c4 + 1)
                                tgt, off = (psoT0, 0) if c4 < 2 else (psoT1, 1024)
                                nc.tensor.matmul(
                                    tgt[0:DK + 1, u - off:w - off],
                                    v_t[:, jb, h, :],
                                    expS_t[:, u:w],
                                    start=(jb == 0),
                                    stop=(jb == min(NB - 1, 4 * c4 + 3)),
                                    skip_group_check=True,
                                )
                            # half 0 of psoT is complete once jb 7 has landed:
                            # evict it early so the next head can reuse psum
                            if jb == 7:
                                nc.vector.tensor_copy(
                                    aU[base : base + DK, pr, 0:1024],
                                    psoT0[0:DK, :],
                                )
                                nc.vector.tensor_copy(
                                    den1[:, 0:1024], psoT0[DK : DK + 1, :]
                                )
                                if h >= HPC - 2:
                                    # last pair: normalize the first half now
                                    # so the output projection isn't gated on
                                    # the end-of-head chain
                                    t, m = emit_norm_chain(h, pr, base, den1, 0, 1024)
                                    t()
                                    self_mul[0] = m
                            # third quarter complete after jb 11: evicting it
                            # early shrinks the end-of-head psum release path
                            if jb == 11:
                                nc.vector.tensor_copy(
                                    aU[base : base + DK, pr, 1024:1536],
                                    psoT1[0:DK, 0:512],
                                )
                                nc.vector.tensor_copy(
                                    den1[:, 1024:1536], psoT1[DK : DK + 1, 0:512]
                                )

                        for jb in range(NB):
                            q0 = P * jb
                            expS_t = spool.tile([P, T], BF16, tag="expS", name="eS")
                            for cc in range(q0 // 1024, 2):
                                lo = max(q0, 1024 * cc)
                                hi = 1024 * (cc + 1)
                                pss = ps_s.tile([P, 1024], F32, tag="pss", name="pss")
                                u = lo
                                while u < hi:
                                    w = min(hi, (u // 512 + 1) * 512)
                                    nc.tensor.matmul(
                                        pss[:, u - 1024 * cc : w - 1024 * cc],
                                        kTz[:, pr, ts(jb, P)],
                                        qT2[:, pr, u:w],
                                        start=True,
                                        stop=True,
                                    )
                                    u = w
                                nc.scalar.activation(
                                    expS_t[:, lo:hi],
                                    pss[:, lo - 1024 * cc : hi - 1024 * cc],
                                    EXP,
                                    scale=float(SCALE / (W8SCALE * W8SCALE)),
                                )
                            # causal mask on the diagonal block
                            nc.vector.tensor_mul(
                                expS_t[:, q0 : q0 + P],
                                expS_t[:, q0 : q0 + P],
                                tri_sb[:],
                            )
                            if jb == 4 and deferred_norm[0] is not None:
                                deferred_norm[0]()
                                deferred_norm[0] = None
                            if jb == 10 and deferred_mul[0] is not None:
                                deferred_mul[0]()
                                deferred_mul[0] = None
                            if jb == 13 and self_mul[0] is not None:
                                self_mul[0]()
                                self_mul[0] = None
                            if pending is not None:
                                emit_attnv(*pending)
                            pending = (jb, expS_t)
                        emit_attnv(*pending)

                        # evict the last quarter; psoT is then free
                        nc.vector.tensor_copy(
                            aU[base : base + DK, pr, 1536:2048],
                            psoT1[0:DK, 512:1024],
                        )
                        nc.vector.tensor_copy(
                            den1[:, 1536:2048], psoT1[DK : DK + 1, 512:1024]
                        )
                        if h == HPC - 2:
                            deferred_norm[0], deferred_mul[0] = emit_norm_chain(
                                h, pr, base, den1, 1024, 2048
                            )
                        elif h == HPC - 1:
                            t, m = emit_norm_chain(h, pr, base, den1, 1024, 2048)
                            t()
                            m()
                        else:
                            deferred_norm[0], deferred_mul[0] = emit_norm_chain(
                                h, pr, base, den1, 0, 2048
                            )
                    for dfr in (deferred_norm, deferred_mul, self_mul):
                        if dfr[0] is not None:
                            dfr[0]()
                            dfr[0] = None

            # ---- phase 3: output projection (partial, transposed) ----
            with (
                tc.tile_pool(name="outp", bufs=6) as ypool,
                tc.tile_pool(name="ps_y", bufs=2, space="PSUM") as ps_y,
            ):
                yTr = yT.rearrange("(ob p) t -> p ob t", p=P)
                for qc4 in range(4):
                    for oblk in range(KB):
                        psy = ps_y.tile(
                            [P, 512], F32, tag=f"psy{oblk % 4}", name="psy"
                        )
                        for cb in range(OB):
                            nc.tensor.matmul(
                                psy[:],
                                wo_sb[:, cb, ts(oblk, P)],
                                aU[:, cb, ts(qc4, 512)],
                                start=(cb == 0),
                                stop=(cb == OB - 1),
                            )
                        ysb = ypool.tile([P, 512], BF16, tag="ysb", name="ysb")
                        if oblk % 2 == 0:
                            nc.vector.tensor_copy(ysb[:], psy[:])
                        else:
                            nc.scalar.copy(ysb[:], psy[:])
                        nc.sync.dma_start(
                            yTr[:, oblk, ts(qc4, 512)], ysb[:]
                        )

    nc.compile()
    return nc


def _host_inputs(x, mask, Wq, bq_v, Wk, bk_v, Wv, bv_v, Wo, bo_v):
    f32 = np.float32
    bf16 = ml_dtypes.bfloat16
    f8 = ml_dtypes.float8_e4m3
    wqT = np.ascontiguousarray(np.asarray(Wq, f32).T) * W8SCALE
    wkT = np.ascontiguousarray(np.asarray(Wk, f32).T) * W8SCALE
    wvT = np.ascontiguousarray(np.asarray(Wv, f32).T).astype(bf16)
    woT = np.ascontiguousarray(np.asarray(Wo, f32).T).astype(bf16)
    # exact v/o bias fold: softmax rows sum to 1, so v+bv adds bv to attn out
    bo_eff = np.asarray(bo_v, f32) + np.asarray(bv_v, f32) @ np.asarray(Wo, f32).T

    # tri[k, q] = 1 where k <= q (causal keep within the diagonal block)
    kk = np.arange(P)
    tri = (kk[:, None] <= kk[None, :]).astype(f32).astype(bf16)

    bqa = np.asarray(bq_v, f32)
    bka = np.asarray(bk_v, f32)
    zebA = np.zeros((P, 1), f32)
    zebA[0:DK] = 1.0

    halves = []
    for half in range(2):
        sl = slice(half * CPC, (half + 1) * CPC)
        # biases carry the fp8 weight pre-scale; exp scale divides it out
        bq_p = np.ascontiguousarray(bqa[sl].reshape(OB, P).T) * W8SCALE
        bk_p = np.ascontiguousarray(bka[sl].reshape(OB, P).T) * W8SCALE
        halves.append(
            {
                "wq8": np.ascontiguousarray(wqT[:, sl]).astype(f8),
                "wk8": np.ascontiguousarray(wkT[:, sl]).astype(f8),
                "wv": np.ascontiguousarray(wvT[:, sl]),
                "wo": np.ascontiguousarray(woT[sl, :]),
                "bq": bq_p,
                "bkzA": bk_p * zebA,
                "bkzB": bk_p * (1.0 - zebA),
                "tri": tri,
            }
        )

    xn = np.asarray(x, f32)
    in_maps = []
    for core in range(8):
        b, half = divmod(core, 2)
        m = dict(halves[half])
        xTb = np.ascontiguousarray(xn[b].T)
        m["xT"] = xTb.astype(bf16)
        m["x8"] = xTb.astype(f8)
        in_maps.append(m)
    return in_maps, bo_eff


def _run(inputs, trace=False):
    if "nc" not in _cache:
        _cache["nc"] = _build()
    nc = _cache["nc"]
    in_maps, bo_eff = _host_inputs(
        inputs["x"], inputs["mask"],
        inputs["Wq"], inputs["bq"], inputs["Wk"], inputs["bk"],
        inputs["Wv"], inputs["bv"], inputs["Wo"], inputs["bo"],
    )
    res = run_bass_kernel_spmd(nc, in_maps, list(range(8)), trace=trace)
    out = np.empty((B, T, C), np.float32)
    for b in range(B):
        yA = res.results[2 * b]["yT"].astype(np.float32)
        yB = res.results[2 * b + 1]["yT"].astype(np.float32)
        out[b] = (yA + yB).T + bo_eff
    return out, res


def kernel(**inputs):
    out, _ = _run(inputs, trace=False)
    return out


# revision 51
# speedup vs baseline: 1.0432x; 1.0199x over previous
"""Multi-head causal attention (B=4, T=2048, C=1024, H=16) on 8 trn2 cores.

Sharding: data-parallel over batch (4) x tensor-parallel over head-halves
(2, Megatron-style): core = 2*b + half computes heads [half*8, half*8+8)
for batch b.  Each core emits a PARTIAL transposed output
yT = Wo_slice.T @ attn_slice.T in [c_out, q] layout; the host sums the
two partials per batch, transposes, and adds bo + bv@Wo.T.

Key kernel tricks vs a naive port:
 - scores run with contraction 128 (full PE stream rate, not the h64
   half-rate): K-proj output is written "zebra"-style into two tiles per
   head pair, one head's 64 channels live, the other's rows zeroed, so
   the [128,128] stationary contracts 64 real + 64 zero rows.
 - attnV is transposed: stationary = v-block [128tok, 65], moving = expS
   [128tok, <=512 q], accumulating psoT[65, 2048] over key blocks with
   per-bank stop flags.  v carries a ones column so psoT row 64 is the
   softmax denominator.
 - normalization: reciprocal of the denom row -> DRAM scratch ->
   stride-0 broadcast DMA back to 64 partitions -> one fused in-place
   multiply on the evicted attn tile.
"""

import numpy as np
import ml_dtypes

import concourse.bass as bass
import concourse.mybir as mybir
import concourse.tile as tile
from concourse import bacc
from concourse.bass import ts
from concourse.bass_utils import run_bass_kernel_spmd

B, T, C, H, DK = 4, 2048, 1024, 16, 64
P = 128
NB = T // P            # 16 key/token blocks
HPC = H // 2           # 8 heads per core
KB = C // P            # 8 input-channel blocks
OB = HPC * DK // P     # 4 output-channel blocks (head pairs) per core
CPC = HPC * DK         # 512 channels per core
SCALE = 1.0 / np.sqrt(DK)
BF16 = mybir.dt.bfloat16
F32 = mybir.dt.float32
F8 = mybir.dt.float8e4
EXP = mybir.ActivationFunctionType.Exp
W8SCALE = 64.0   # fp8 weight pre-scale (keeps w out of subnormal range)

_cache = {}


def _build():
    nc = bacc.Bacc("TRN2", target_bir_lowering=False, debug=False)

    xT = nc.dram_tensor("xT", [C, T], BF16, kind="ExternalInput").ap()
    x8 = nc.dram_tensor("x8", [C, T], F8, kind="ExternalInput").ap()
    wq8 = nc.dram_tensor("wq8", [C, CPC], F8, kind="ExternalInput").ap()
    wk8 = nc.dram_tensor("wk8", [C, CPC], F8, kind="ExternalInput").ap()
    wv = nc.dram_tensor("wv", [C, CPC], BF16, kind="ExternalInput").ap()
    wo = nc.dram_tensor("wo", [CPC, C], BF16, kind="ExternalInput").ap()
    bq = nc.dram_tensor("bq", [P, OB], F32, kind="ExternalInput").ap()
    bkzA = nc.dram_tensor("bkzA", [P, OB], F32, kind="ExternalInput").ap()
    bkzB = nc.dram_tensor("bkzB", [P, OB], F32, kind="ExternalInput").ap()
    tri = nc.dram_tensor("tri", [P, P], BF16, kind="ExternalInput").ap()
    yT = nc.dram_tensor("yT", [C, T], BF16, kind="ExternalOutput").ap()

    with tile.TileContext(nc) as tc:
        with (
            tc.tile_pool(name="const", bufs=1) as cpool,
            tc.tile_pool(name="acc", bufs=1) as apool,
            tc.tile_pool(name="dram", bufs=1, space="DRAM") as dpool,
        ):
            tri_sb = cpool.tile([P, P], BF16)
            bq_sb = cpool.tile([P, OB], F32)
            bkzA_sb = cpool.tile([P, OB], F32)
            bkzB_sb = cpool.tile([P, OB], F32)
            z01 = cpool.tile([P, 1], F32)
            z10 = cpool.tile([P, 1], F32)
            wo_sb = cpool.tile([P, OB, C], BF16)
            nc.vector.memset(z01[0:DK, :], 1.0)
            nc.vector.memset(z01[DK:P, :], 0.0)
            nc.vector.memset(z10[0:DK, :], 0.0)
            nc.vector.memset(z10[DK:P, :], 1.0)

            aU = apool.tile([P, OB, T], BF16)     # attn.T, normalized in place
            recB = apool.tile([P, OB, T], BF16)   # 1/denom broadcast per head
            dscr = dpool.tile([HPC, T], F32)      # DRAM scratch: denom rows
            rscr = dpool.tile([HPC, T], BF16)     # DRAM scratch: recip rows

            with tc.tile_pool(name="qkv", bufs=1) as qkv:
                qT2 = qkv.tile([P, OB, T], BF16)        # q.T, dense head pairs
                kTzA = qkv.tile([P, OB, T], BF16)       # k.T, even head live
                kTzB = qkv.tile([P, OB, T], BF16)       # k.T, odd head live
                v_t = qkv.tile([P, NB, HPC, DK + 1], BF16)
                nc.vector.memset(v_t[:, :, :, DK : DK + 1], 1.0)

                # ---- phase 1: q/k/v projections ----
                with (
                    tc.tile_pool(name="xtw", bufs=1) as xtw,
                    tc.tile_pool(name="pacc", bufs=2, space="PSUM") as pacc,
                ):
                    xT_sb = xtw.tile([P, KB, T], BF16)
                    x8_sb = xtw.tile([P, KB, T], F8)
                    wq8_sb = xtw.tile([P, KB, CPC], F8)
                    wk8_sb = xtw.tile([P, KB, CPC], F8)
                    wv_sb = xtw.tile([P, KB, CPC], BF16)

                    wqr = wq8.rearrange("(kb p) o -> p kb o", p=P)
                    x8r = x8.rearrange("(kb p) t -> p kb t", p=P)
                    xTr = xT.rearrange("(kb p) t -> p kb t", p=P)
                    # kb-half granularity on the critical first loads so the
                    # first Q matmul starts as early as possible
                    nc.gpsimd.dma_start(wq8_sb[:, 0:4, :], wqr[:, 0:4, :])
                    nc.gpsimd.dma_start(x8_sb[:, 0:4, 0:512], x8r[:, 0:4, 0:512])
                    nc.gpsimd.dma_start(wq8_sb[:, 4:8, :], wqr[:, 4:8, :])
                    nc.gpsimd.dma_start(x8_sb[:, 4:8, 0:512], x8r[:, 4:8, 0:512])
                    nc.gpsimd.dma_start(x8_sb[:, :, 512:1024], x8r[:, :, 512:1024])
                    nc.gpsimd.dma_start(bq_sb[:], bq)
                    nc.gpsimd.dma_start(bkzA_sb[:], bkzA)
                    nc.gpsimd.dma_start(bkzB_sb[:], bkzB)
                    nc.gpsimd.dma_start(tri_sb[:], tri)
                    for tc4 in range(2, 4):
                        nc.gpsimd.dma_start(
                            x8_sb[:, :, ts(tc4, 512)], x8r[:, :, ts(tc4, 512)]
                        )
                    nc.gpsimd.dma_start(
                        wv_sb[:], wv.rearrange("(kb p) o -> p kb o", p=P)
                    )
                    for tc4 in range(4):
                        nc.gpsimd.dma_start(
                            xT_sb[:, :, ts(tc4, 512)], xTr[:, :, ts(tc4, 512)]
                        )
                    nc.gpsimd.dma_start(
                        wk8_sb[:], wk8.rearrange("(kb p) o -> p kb o", p=P)
                    )
                    nc.gpsimd.dma_start(
                        wo_sb[:], wo.rearrange("(cb p) o -> p cb o", p=P)
                    )

                    # Q projection -> qT2 [d(pair), pair, q]; fp8 DoubleRow
                    # contracts two 128-channel slabs per instruction
                    for tc4 in range(4):
                        accs = [
                            pacc.tile([P, 512], F32, tag=f"acc{ob}", name=f"qa{ob}")
                            for ob in range(OB)
                        ]
                        for kb2 in range(KB // 2):
                            for ob in range(OB):
                                nc.tensor.matmul(
                                    accs[ob][:],
                                    wq8_sb[:, 2 * kb2 : 2 * kb2 + 2, ts(ob, P)],
                                    x8_sb[:, 2 * kb2 : 2 * kb2 + 2, ts(tc4, 512)],
                                    start=(kb2 == 0),
                                    stop=(kb2 == KB // 2 - 1),
                                    perf_mode=mybir.MatmulPerfMode.DoubleRow,
                                )
                        for ob in range(OB):
                            nc.vector.tensor_scalar_add(
                                qT2[:, ob, ts(tc4, 512)],
                                accs[ob][:],
                                bq_sb[:, ob : ob + 1],
                            )

                    # V projection -> v_t [tok, tb, h, d] + ones column
                    for tb in range(NB):
                        acc = pacc.tile(
                            [P, 512], F32, tag=f"acc{tb % 4}", name="va"
                        )
                        for kb in range(KB):
                            nc.tensor.matmul(
                                acc[:],
                                xT_sb[:, kb, ts(tb, P)],
                                wv_sb[:, kb, :],
                                start=(kb == 0),
                                stop=(kb == KB - 1),
                            )
                        # eviction on ACT: DVE is the busier engine here
                        nc.scalar.copy(
                            v_t[:, tb, :, 0:DK],
                            acc[:].rearrange("p (h e) -> p h e", e=DK),
                        )

                    # K projection -> zebra tiles (last, so attention's ACT
                    # work can begin as soon as K lands)
                    for tc4 in range(4):
                        accs = [
                            pacc.tile([P, 512], F32, tag=f"acc{ob}", name=f"ka{ob}")
                            for ob in range(OB)
                        ]
                        for kb2 in range(KB // 2):
                            for ob in range(OB):
                                nc.tensor.matmul(
                                    accs[ob][:],
                                    wk8_sb[:, 2 * kb2 : 2 * kb2 + 2, ts(ob, P)],
                                    x8_sb[:, 2 * kb2 : 2 * kb2 + 2, ts(tc4, 512)],
                                    start=(kb2 == 0),
                                    stop=(kb2 == KB // 2 - 1),
                                    perf_mode=mybir.MatmulPerfMode.DoubleRow,
                                )
                        for ob in range(OB):
                            nc.vector.tensor_scalar(
                                kTzA[:, ob, ts(tc4, 512)],
                                accs[ob][:],
                                z01[:],
                                bkzA_sb[:, ob : ob + 1],
                                mybir.AluOpType.mult,
                                mybir.AluOpType.add,
                            )
                            # second zebra eviction on ACT (Identity with
                            # per-partition scale+bias) to halve DVE drain
                            nc.scalar.activation(
                                kTzB[:, ob, ts(tc4, 512)],
                                accs[ob][:],
                                mybir.ActivationFunctionType.Identity,
                                bias=bkzB_sb[:, ob : ob + 1],
                                scale=z10[:],
                            )

                # ---- phase 2: attention per head ----
                with (
                    tc.tile_pool(name="expp", bufs=3) as spool,
                    tc.tile_pool(name="recp", bufs=2) as rpool,
                    tc.tile_pool(name="ps_s", bufs=2, space="PSUM") as ps_s,
                    tc.tile_pool(name="ps_o", bufs=1, space="PSUM") as ps_o,
                ):
                    deferred_norm = [None]
                    deferred_mul = [None]
                    self_mul = [None]

                    def emit_norm_chain(h, pr, base, den1, lo, hi):
                        """Denominator -> reciprocal -> broadcast -> multiply
                        for q columns [lo, hi).  Returns (tail, mul) emitters
                        so callers can defer each past its DMA latency."""
                        nr = (hi - lo) // P
                        nc.sync.dma_start(dscr[h : h + 1, lo:hi], den1[:, lo:hi])
                        denT = rpool.tile([NB, P], F32, tag="denT", name="denT")
                        nc.sync.dma_start(
                            denT[0:nr, :],
                            dscr[h : h + 1, lo:hi].rearrange(
                                "o (p c) -> (o p) c", c=P
                            ),
                        )

                        def tail():
                            recT = rpool.tile([NB, P], BF16, tag="recT", name="recT")
                            with nc.allow_low_precision(reason="softmax recip"):
                                nc.vector.reciprocal(recT[0:nr, :], denT[0:nr, :])
                            nc.sync.dma_start(
                                rscr[h : h + 1, lo:hi].rearrange(
                                    "o (p c) -> (o p) c", c=P
                                ),
                                recT[0:nr, :],
                            )
                            nc.gpsimd.dma_start(
                                recB[base : base + DK, pr, lo:hi],
                                rscr[h : h + 1, lo:hi].partition_broadcast(DK),
                            )

                        def mul():
                            nc.vector.tensor_mul(
                                aU[base : base + DK, pr, lo:hi],
                                aU[base : base + DK, pr, lo:hi],
                                recB[base : base + DK, pr, lo:hi],
                            )

                        return tail, mul

                    for h in range(HPC):
                        pr, hp = h // 2, h % 2
                        kTz = kTzA if hp == 0 else kTzB
                        base = hp * DK
                        psoT0 = ps_o.tile([P, 1024], F32, tag="pso0", name="psoT0")
                        psoT1 = ps_o.tile([P, 1024], F32, tag="pso1", name="psoT1")
                        den1 = rpool.tile([1, T], F32, tag="den1", name="den1")
                        pending = None

                        def emit_attnv(jb, expS_t, col0=0):
                            q0 = P * jb
                            for c4 in range(q0 // 512, 4):
                                u = max(q0, 512 * c4)
                                w = 512 * (c4 + 1)
                                tgt, off = (psoT0, 0) if c4 < 2 else (psoT1, 1024)
                                nc.tensor.matmul(
                                    tgt[0:DK + 1, u - off:w - off],
                                    v_t[:, jb, h, :],
                                    expS_t[:, u - col0:w - col0],
                                    start=(jb == 0),
                                    stop=(jb == min(NB - 1, 4 * c4 + 3)),
                                    skip_group_check=True,
                                )
                            # half 0 of psoT is complete once jb 7 has landed:
                            # evict it early so the next head can reuse psum
                            if jb == 7:
                                nc.vector.tensor_copy(
                                    aU[base : base + DK, pr, 0:1024],
                                    psoT0[0:DK, :],
                                )
                                nc.vector.tensor_copy(
                                    den1[:, 0:1024], psoT0[DK : DK + 1, :]
                                )
                                if h >= HPC - 2:
                                    # last pair: normalize the first half now
                                    # so the output projection isn't gated on
                                    # the end-of-head chain
                                    t, m = emit_norm_chain(h, pr, base, den1, 0, 1024)
                                    t()
                                    self_mul[0] = m
                            # third quarter complete after jb 11: evicting it
                            # early shrinks the end-of-head psum release path
                            if jb == 11:
                                nc.vector.tensor_copy(
                                    aU[base : base + DK, pr, 1024:1536],
                                    psoT1[0:DK, 0:512],
                                )
                                nc.vector.tensor_copy(
                                    den1[:, 1024:1536], psoT1[DK : DK + 1, 0:512]
                                )

                        def jb_hooks(jb):
                            if jb == 4 and deferred_norm[0] is not None:
                                deferred_norm[0]()
                                deferred_norm[0] = None
                            if jb == 10 and deferred_mul[0] is not None:
                                deferred_mul[0]()
                                deferred_mul[0] = None
                            if jb == 13 and self_mul[0] is not None:
                                self_mul[0]()
                                self_mul[0] = None

                        for jb in range(12):
                            q0 = P * jb
                            expS_t = spool.tile([P, T], BF16, tag="expS", name="eS")
                            for cc in range(q0 // 1024, 2):
                                lo = max(q0, 1024 * cc)
                                hi = 1024 * (cc + 1)
                                pss = ps_s.tile([P, 1024], F32, tag="pss", name="pss")
                                u = lo
                                while u < hi:
                                    w = min(hi, (u // 512 + 1) * 512)
                                    nc.tensor.matmul(
                                        pss[:, u - 1024 * cc : w - 1024 * cc],
                                        kTz[:, pr, ts(jb, P)],
                                        qT2[:, pr, u:w],
                                        start=True,
                                        stop=True,
                                    )
                                    u = w
                                nc.scalar.activation(
                                    expS_t[:, lo:hi],
                                    pss[:, lo - 1024 * cc : hi - 1024 * cc],
                                    EXP,
                                    scale=float(SCALE / (W8SCALE * W8SCALE)),
                                )
                            # causal mask on the diagonal block
                            nc.vector.tensor_mul(
                                expS_t[:, q0 : q0 + P],
                                expS_t[:, q0 : q0 + P],
                                tri_sb[:],
                            )
                            jb_hooks(jb)
                            if pending is not None:
                                emit_attnv(*pending)
                            pending = (jb, expS_t, 0)
                        # jbs 12..15 are narrow: pack pairs into one psum tile
                        # (bank-aligned) and drain each pair with ONE exp
                        for ja in (12, 14):
                            pss = ps_s.tile([P, 1024], F32, tag="pss", name="pss")
                            expS2 = spool.tile(
                                [P, 1024], BF16, tag="expSL", name="eSL"
                            )
                            locs = {ja: 0, ja + 1: 512}
                            span = 512 + (T - P * (ja + 1))
                            for j in (ja, ja + 1):
                                q0j = P * j
                                nc.tensor.matmul(
                                    pss[:, locs[j] : locs[j] + T - q0j],
                                    kTz[:, pr, ts(j, P)],
                                    qT2[:, pr, q0j:T],
                                    start=True,
                                    stop=True,
                                )
                            nc.scalar.activation(
                                expS2[:, 0:span],
                                pss[:, 0:span],
                                EXP,
                                scale=float(SCALE / (W8SCALE * W8SCALE)),
                            )
                            for j in (ja, ja + 1):
                                nc.vector.tensor_mul(
                                    expS2[:, locs[j] : locs[j] + P],
                                    expS2[:, locs[j] : locs[j] + P],
                                    tri_sb[:],
                                )
                                jb_hooks(j)
                                if pending is not None:
                                    emit_attnv(*pending)
                                pending = (j, expS2, P * j - locs[j])
                        emit_attnv(*pending)

                        # evict the last quarter; psoT is then free
                        nc.vector.tensor_copy(
                            aU[base : base + DK, pr, 1536:2048],
                            psoT1[0:DK, 512:1024],
                        )
                        nc.vector.tensor_copy(
                            den1[:, 1536:2048], psoT1[DK : DK + 1, 512:1024]
                        )
                        if h == HPC - 2:
                            deferred_norm[0], deferred_mul[0] = emit_norm_chain(
                                h, pr, base, den1, 1024, 2048
                            )
                        elif h == HPC - 1:
                            t, m = emit_norm_chain(h, pr, base, den1, 1024, 2048)
                            t()
                            m()
                        else:
                            deferred_norm[0], deferred_mul[0] = emit_norm_chain(
                                h, pr, base, den1, 0, 2048
                            )
                    for dfr in (deferred_norm, deferred_mul, self_mul):
                        if dfr[0] is not None:
                            dfr[0]()
                            dfr[0] = None

            # ---- phase 3: output projection (partial, transposed) ----
            with (
                tc.tile_pool(name="outp", bufs=6) as ypool,
                tc.tile_pool(name="ps_y", bufs=2, space="PSUM") as ps_y,
            ):
                yTr = yT.rearrange("(ob p) t -> p ob t", p=P)
                for qc4 in range(4):
                    for oblk in range(KB):
                        psy = ps_y.tile(
                            [P, 512], F32, tag=f"psy{oblk % 4}", name="psy"
                        )
                        for cb in range(OB):
                            nc.tensor.matmul(
                                psy[:],
                                wo_sb[:, cb, ts(oblk, P)],
                                aU[:, cb, ts(qc4, 512)],
                                start=(cb == 0),
                                stop=(cb == OB - 1),
                            )
                        ysb = ypool.tile([P, 512], BF16, tag="ysb", name="ysb")
                        if oblk % 2 == 0:
                            nc.vector.tensor_copy(ysb[:], psy[:])
                        else:
                            nc.scalar.copy(ysb[:], psy[:])
                        nc.sync.dma_start(
                            yTr[:, oblk, ts(qc4, 512)], ysb[:]
                        )

    nc.compile()
    return nc


def _host_inputs(x, mask, Wq, bq_v, Wk, bk_v, Wv, bv_v, Wo, bo_v):
    f32 = np.float32
    bf16 = ml_dtypes.bfloat16
    f8 = ml_dtypes.float8_e4m3
    wqT = np.ascontiguousarray(np.asarray(Wq, f32).T) * W8SCALE
    wkT = np.ascontiguousarray(np.asarray(Wk, f32).T) * W8SCALE
    wvT = np.ascontiguousarray(np.asarray(Wv, f32).T).astype(bf16)
    woT = np.ascontiguousarray(np.asarray(Wo, f32).T).astype(bf16)
    # exact v/o bias fold: softmax rows sum to 1, so v+bv adds bv to attn out
    bo_eff = np.asarray(bo_v, f32) + np.asarray(bv_v, f32) @ np.asarray(Wo, f32).T

    # tri[k, q] = 1 where k <= q (causal keep within the diagonal block)
    kk = np.arange(P)
    tri = (kk[:, None] <= kk[None, :]).astype(f32).astype(bf16)

    bqa = np.asarray(bq_v, f32)
    bka = np.asarray(bk_v, f32)
    zebA = np.zeros((P, 1), f32)
    zebA[0:DK] = 1.0

    halves = []
    for half in range(2):
        sl = slice(half * CPC, (half + 1) * CPC)
        # biases carry the fp8 weight pre-scale; exp scale divides it out
        bq_p = np.ascontiguousarray(bqa[sl].reshape(OB, P).T) * W8SCALE
        bk_p = np.ascontiguousarray(bka[sl].reshape(OB, P).T) * W8SCALE
        halves.append(
            {
                "wq8": np.ascontiguousarray(wqT[:, sl]).astype(f8),
                "wk8": np.ascontiguousarray(wkT[:, sl]).astype(f8),
                "wv": np.ascontiguousarray(wvT[:, sl]),
                "wo": np.ascontiguousarray(woT[sl, :]),
                "bq": bq_p,
                "bkzA": bk_p * zebA,
                "bkzB": bk_p * (1.0 - zebA),
                "tri": tri,
            }
        )

    xn = np.asarray(x, f32)
    in_maps = []
    for core in range(8):
        b, half = divmod(core, 2)
        m = dict(halves[half])
        xTb = np.ascontiguousarray(xn[b].T)
        m["xT"] = xTb.astype(bf16)
        m["x8"] = xTb.astype(f8)
        in_maps.append(m)
    return in_maps, bo_eff


def _run(inputs, trace=False):
    if "nc" not in _cache:
        _cache["nc"] = _build()
    nc = _cache["nc"]
    in_maps, bo_eff = _host_inputs(
        inputs["x"], inputs["mask"],
        inputs["Wq"], inputs["bq"], inputs["Wk"], inputs["bk"],
        inputs["Wv"], inputs["bv"], inputs["Wo"], inputs["bo"],
    )
    res = run_bass_kernel_spmd(nc, in_maps, list(range(8)), trace=trace)
    out = np.empty((B, T, C), np.float32)
    for b in range(B):
        yA = res.results[2 * b]["yT"].astype(np.float32)
        yB = res.results[2 * b + 1]["yT"].astype(np.float32)
        out[b] = (yA + yB).T + bo_eff
    return out, res


def kernel(**inputs):
    out, _ = _run(inputs, trace=False)
    return out


# revision 52
# speedup vs baseline: 1.0583x; 1.0145x over previous
"""Multi-head causal attention (B=4, T=2048, C=1024, H=16) on 8 trn2 cores.

Sharding: data-parallel over batch (4) x tensor-parallel over head-halves
(2, Megatron-style): core = 2*b + half computes heads [half*8, half*8+8)
for batch b.  Each core emits a PARTIAL transposed output
yT = Wo_slice.T @ attn_slice.T in [c_out, q] layout; the host sums the
two partials per batch, transposes, and adds bo + bv@Wo.T.

Key kernel tricks vs a naive port:
 - scores run with contraction 128 (full PE stream rate, not the h64
   half-rate): K-proj output is written "zebra"-style into two tiles per
   head pair, one head's 64 channels live, the other's rows zeroed, so
   the [128,128] stationary contracts 64 real + 64 zero rows.
 - attnV is transposed: stationary = v-block [128tok, 65], moving = expS
   [128tok, <=512 q], accumulating psoT[65, 2048] over key blocks with
   per-bank stop flags.  v carries a ones column so psoT row 64 is the
   softmax denominator.
 - normalization: reciprocal of the denom row -> DRAM scratch ->
   stride-0 broadcast DMA back to 64 partitions -> one fused in-place
   multiply on the evicted attn tile.
"""

import numpy as np
import ml_dtypes

import concourse.bass as bass
import concourse.mybir as mybir
import concourse.tile as tile
from concourse import bacc
from concourse.bass import ts
from concourse.bass_utils import run_bass_kernel_spmd

B, T, C, H, DK = 4, 2048, 1024, 16, 64
P = 128
NB = T // P            # 16 key/token blocks
HPC = H // 2           # 8 heads per core
KB = C // P            # 8 input-channel blocks
OB = HPC * DK // P     # 4 output-channel blocks (head pairs) per core
CPC = HPC * DK         # 512 channels per core
SCALE = 1.0 / np.sqrt(DK)
BF16 = mybir.dt.bfloat16
F32 = mybir.dt.float32
F8 = mybir.dt.float8e4
EXP = mybir.ActivationFunctionType.Exp
W8SCALE = 64.0   # fp8 weight pre-scale (keeps w out of subnormal range)

_cache = {}


def _build():
    nc = bacc.Bacc("TRN2", target_bir_lowering=False, debug=False)

    xT = nc.dram_tensor("xT", [C, T], BF16, kind="ExternalInput").ap()
    x8 = nc.dram_tensor("x8", [C, T], F8, kind="ExternalInput").ap()
    wq8 = nc.dram_tensor("wq8", [C, CPC], F8, kind="ExternalInput").ap()
    wk8 = nc.dram_tensor("wk8", [C, CPC], F8, kind="ExternalInput").ap()
    wv = nc.dram_tensor("wv", [C, CPC], BF16, kind="ExternalInput").ap()
    wo = nc.dram_tensor("wo", [CPC, C], BF16, kind="ExternalInput").ap()
    bq = nc.dram_tensor("bq", [P, OB], F32, kind="ExternalInput").ap()
    bkzA = nc.dram_tensor("bkzA", [P, OB], F32, kind="ExternalInput").ap()
    bkzB = nc.dram_tensor("bkzB", [P, OB], F32, kind="ExternalInput").ap()
    tri = nc.dram_tensor("tri", [P, P], BF16, kind="ExternalInput").ap()
    yT = nc.dram_tensor("yT", [C, T], BF16, kind="ExternalOutput").ap()

    with tile.TileContext(nc) as tc:
        with (
            tc.tile_pool(name="const", bufs=1) as cpool,
            tc.tile_pool(name="acc", bufs=1) as apool,
            tc.tile_pool(name="dram", bufs=1, space="DRAM") as dpool,
        ):
            tri_sb = cpool.tile([P, P], BF16)
            bq_sb = cpool.tile([P, OB], F32)
            bkzA_sb = cpool.tile([P, OB], F32)
            bkzB_sb = cpool.tile([P, OB], F32)
            z01 = cpool.tile([P, 1], F32)
            z10 = cpool.tile([P, 1], F32)
            wo_sb = cpool.tile([P, OB, C], BF16)
            nc.vector.memset(z01[0:DK, :], 1.0)
            nc.vector.memset(z01[DK:P, :], 0.0)
            nc.vector.memset(z10[0:DK, :], 0.0)
            nc.vector.memset(z10[DK:P, :], 1.0)

            aU = apool.tile([P, OB, T], BF16)     # attn.T, normalized in place
            recB = apool.tile([P, OB, T], BF16)   # 1/denom broadcast per head
            dscr = dpool.tile([HPC, T], F32)      # DRAM scratch: denom rows
            rscr = dpool.tile([HPC, T], BF16)     # DRAM scratch: recip rows

            with tc.tile_pool(name="qkv", bufs=1) as qkv:
                qT2 = qkv.tile([P, OB, T], BF16)        # q.T, dense head pairs
                kTzA = qkv.tile([P, OB, T], BF16)       # k.T, even head live
                kTzB = qkv.tile([P, OB, T], BF16)       # k.T, odd head live
                v_t = qkv.tile([P, NB, HPC, DK + 1], BF16)
                nc.vector.memset(v_t[:, :, :, DK : DK + 1], 1.0)

                # ---- phase 1: q/k/v projections ----
                with (
                    tc.tile_pool(name="xtw", bufs=1) as xtw,
                    tc.tile_pool(name="pacc", bufs=2, space="PSUM") as pacc,
                ):
                    xT_sb = xtw.tile([P, KB, T], BF16)
                    x8_sb = xtw.tile([P, KB, T], F8)
                    wq8_sb = xtw.tile([P, KB, CPC], F8)
                    wk8_sb = xtw.tile([P, KB, CPC], F8)
                    wv_sb = xtw.tile([P, KB, CPC], BF16)

                    wqr = wq8.rearrange("(kb p) o -> p kb o", p=P)
                    x8r = x8.rearrange("(kb p) t -> p kb t", p=P)
                    xTr = xT.rearrange("(kb p) t -> p kb t", p=P)
                    # kb-half granularity on the critical first loads so the
                    # first Q matmul starts as early as possible
                    nc.gpsimd.dma_start(wq8_sb[:, 0:4, :], wqr[:, 0:4, :])
                    nc.gpsimd.dma_start(x8_sb[:, 0:4, 0:512], x8r[:, 0:4, 0:512])
                    nc.gpsimd.dma_start(wq8_sb[:, 4:8, :], wqr[:, 4:8, :])
                    nc.gpsimd.dma_start(x8_sb[:, 4:8, 0:512], x8r[:, 4:8, 0:512])
                    nc.gpsimd.dma_start(x8_sb[:, :, 512:1024], x8r[:, :, 512:1024])
                    nc.gpsimd.dma_start(bq_sb[:], bq)
                    nc.gpsimd.dma_start(bkzA_sb[:], bkzA)
                    nc.gpsimd.dma_start(bkzB_sb[:], bkzB)
                    nc.gpsimd.dma_start(tri_sb[:], tri)
                    for tc4 in range(2, 4):
                        nc.gpsimd.dma_start(
                            x8_sb[:, :, ts(tc4, 512)], x8r[:, :, ts(tc4, 512)]
                        )
                    nc.gpsimd.dma_start(
                        wv_sb[:], wv.rearrange("(kb p) o -> p kb o", p=P)
                    )
                    for tc4 in range(4):
                        nc.gpsimd.dma_start(
                            xT_sb[:, :, ts(tc4, 512)], xTr[:, :, ts(tc4, 512)]
                        )
                    nc.gpsimd.dma_start(
                        wk8_sb[:], wk8.rearrange("(kb p) o -> p kb o", p=P)
                    )
                    nc.gpsimd.dma_start(
                        wo_sb[:], wo.rearrange("(cb p) o -> p cb o", p=P)
                    )

                    # Q projection -> qT2 [d(pair), pair, q]; fp8 DoubleRow
                    # contracts two 128-channel slabs per instruction
                    for tc4 in range(4):
                        accs = [
                            pacc.tile([P, 512], F32, tag=f"acc{ob}", name=f"qa{ob}")
                            for ob in range(OB)
                        ]
                        for kb2 in range(KB // 2):
                            for ob in range(OB):
                                nc.tensor.matmul(
                                    accs[ob][:],
                                    wq8_sb[:, 2 * kb2 : 2 * kb2 + 2, ts(ob, P)],
                                    x8_sb[:, 2 * kb2 : 2 * kb2 + 2, ts(tc4, 512)],
                                    start=(kb2 == 0),
                                    stop=(kb2 == KB // 2 - 1),
                                    perf_mode=mybir.MatmulPerfMode.DoubleRow,
                                )
                        for ob in range(OB):
                            nc.vector.tensor_scalar_add(
                                qT2[:, ob, ts(tc4, 512)],
                                accs[ob][:],
                                bq_sb[:, ob : ob + 1],
                            )

                    # V projection -> v_t [tok, tb, h, d] + ones column
                    for tb in range(NB):
                        acc = pacc.tile(
                            [P, 512], F32, tag=f"acc{tb % 4}", name="va"
                        )
                        for kb in range(KB):
                            nc.tensor.matmul(
                                acc[:],
                                xT_sb[:, kb, ts(tb, P)],
                                wv_sb[:, kb, :],
                                start=(kb == 0),
                                stop=(kb == KB - 1),
                            )
                        # eviction on ACT: DVE is the busier engine here
                        nc.scalar.copy(
                            v_t[:, tb, :, 0:DK],
                            acc[:].rearrange("p (h e) -> p h e", e=DK),
                        )

                    # K projection -> zebra tiles (last, so attention's ACT
                    # work can begin as soon as K lands)
                    for tc4 in range(4):
                        accs = [
                            pacc.tile([P, 512], F32, tag=f"acc{ob}", name=f"ka{ob}")
                            for ob in range(OB)
                        ]
                        for kb2 in range(KB // 2):
                            for ob in range(OB):
                                nc.tensor.matmul(
                                    accs[ob][:],
                                    wk8_sb[:, 2 * kb2 : 2 * kb2 + 2, ts(ob, P)],
                                    x8_sb[:, 2 * kb2 : 2 * kb2 + 2, ts(tc4, 512)],
                                    start=(kb2 == 0),
                                    stop=(kb2 == KB // 2 - 1),
                                    perf_mode=mybir.MatmulPerfMode.DoubleRow,
                                )
                        for ob in range(OB):
                            nc.vector.tensor_scalar(
                                kTzA[:, ob, ts(tc4, 512)],
                                accs[ob][:],
                                z01[:],
                                bkzA_sb[:, ob : ob + 1],
                                mybir.AluOpType.mult,
                                mybir.AluOpType.add,
                            )
                            # second zebra eviction on ACT (Identity with
                            # per-partition scale+bias) to halve DVE drain
                            nc.scalar.activation(
                                kTzB[:, ob, ts(tc4, 512)],
                                accs[ob][:],
                                mybir.ActivationFunctionType.Identity,
                                bias=bkzB_sb[:, ob : ob + 1],
                                scale=z10[:],
                            )

                # ---- phase 2: attention per head ----
                with (
                    tc.tile_pool(name="expp", bufs=4) as spool,
                    tc.tile_pool(name="recp", bufs=2) as rpool,
                    tc.tile_pool(name="ps_s", bufs=2, space="PSUM") as ps_s,
                    tc.tile_pool(name="ps_o", bufs=1, space="PSUM") as ps_o,
                ):
                    deferred_norm = [None]
                    deferred_mul = [None]
                    self_mul = [None]

                    def emit_norm_chain(h, pr, base, den1, lo, hi):
                        """Denominator -> reciprocal -> broadcast -> multiply
                        for q columns [lo, hi).  Returns (tail, mul) emitters
                        so callers can defer each past its DMA latency."""
                        nr = (hi - lo) // P
                        nc.sync.dma_start(dscr[h : h + 1, lo:hi], den1[:, lo:hi])
                        denT = rpool.tile([NB, P], F32, tag="denT", name="denT")
                        nc.sync.dma_start(
                            denT[0:nr, :],
                            dscr[h : h + 1, lo:hi].rearrange(
                                "o (p c) -> (o p) c", c=P
                            ),
                        )

                        def tail():
                            recT = rpool.tile([NB, P], BF16, tag="recT", name="recT")
                            with nc.allow_low_precision(reason="softmax recip"):
                                nc.vector.reciprocal(recT[0:nr, :], denT[0:nr, :])
                            nc.sync.dma_start(
                                rscr[h : h + 1, lo:hi].rearrange(
                                    "o (p c) -> (o p) c", c=P
                                ),
                                recT[0:nr, :],
                            )
                            nc.gpsimd.dma_start(
                                recB[base : base + DK, pr, lo:hi],
                                rscr[h : h + 1, lo:hi].partition_broadcast(DK),
                            )

                        def mul():
                            nc.vector.tensor_mul(
                                aU[base : base + DK, pr, lo:hi],
                                aU[base : base + DK, pr, lo:hi],
                                recB[base : base + DK, pr, lo:hi],
                            )

                        return tail, mul

                    for h in range(HPC):
                        pr, hp = h // 2, h % 2
                        kTz = kTzA if hp == 0 else kTzB
                        base = hp * DK
                        psoT0 = ps_o.tile([P, 1024], F32, tag="pso0", name="psoT0")
                        psoT1 = ps_o.tile([P, 1024], F32, tag="pso1", name="psoT1")
                        den1 = rpool.tile([1, T], F32, tag="den1", name="den1")
                        pending = None

                        def emit_attnv(jb, expS_t, col0=0):
                            q0 = P * jb
                            for c4 in range(q0 // 512, 4):
                                u = max(q0, 512 * c4)
                                w = 512 * (c4 + 1)
                                tgt, off = (psoT0, 0) if c4 < 2 else (psoT1, 1024)
                                nc.tensor.matmul(
                                    tgt[0:DK + 1, u - off:w - off],
                                    v_t[:, jb, h, :],
                                    expS_t[:, u - col0:w - col0],
                                    start=(jb == 0),
                                    stop=(jb == min(NB - 1, 4 * c4 + 3)),
                                    skip_group_check=True,
                                )
                            # half 0 of psoT is complete once jb 7 has landed:
                            # evict it early so the next head can reuse psum
                            if jb == 7:
                                nc.vector.tensor_copy(
                                    aU[base : base + DK, pr, 0:1024],
                                    psoT0[0:DK, :],
                                )
                                nc.vector.tensor_copy(
                                    den1[:, 0:1024], psoT0[DK : DK + 1, :]
                                )
                                if h >= HPC - 2:
                                    # last pair: normalize the first half now
                                    # so the output projection isn't gated on
                                    # the end-of-head chain
                                    t, m = emit_norm_chain(h, pr, base, den1, 0, 1024)
                                    t()
                                    self_mul[0] = m
                            # third quarter complete after jb 11: evicting it
                            # early shrinks the end-of-head psum release path
                            if jb == 11:
                                nc.vector.tensor_copy(
                                    aU[base : base + DK, pr, 1024:1536],
                                    psoT1[0:DK, 0:512],
                                )
                                nc.vector.tensor_copy(
                                    den1[:, 1024:1536], psoT1[DK : DK + 1, 0:512]
                                )

                        def jb_hooks(jb):
                            if jb == 4 and deferred_norm[0] is not None:
                                deferred_norm[0]()
                                deferred_norm[0] = None
                            if jb == 10 and deferred_mul[0] is not None:
                                deferred_mul[0]()
                                deferred_mul[0] = None
                            if jb == 13 and self_mul[0] is not None:
                                self_mul[0]()
                                self_mul[0] = None

                        for jb in range(12):
                            q0 = P * jb
                            expS_t = spool.tile([P, T], BF16, tag="expS", name="eS")
                            for cc in range(q0 // 1024, 2):
                                lo = max(q0, 1024 * cc)
                                hi = 1024 * (cc + 1)
                                pss = ps_s.tile([P, 1024], F32, tag="pss", name="pss")
                                u = lo
                                while u < hi:
                                    w = min(hi, (u // 512 + 1) * 512)
                                    nc.tensor.matmul(
                                        pss[:, u - 1024 * cc : w - 1024 * cc],
                                        kTz[:, pr, ts(jb, P)],
                                        qT2[:, pr, u:w],
                                        start=True,
                                        stop=True,
                                    )
                                    u = w
                                nc.scalar.activation(
                                    expS_t[:, lo:hi],
                                    pss[:, lo - 1024 * cc : hi - 1024 * cc],
                                    EXP,
                                    scale=float(SCALE / (W8SCALE * W8SCALE)),
                                )
                            # causal mask on the diagonal block
                            nc.vector.tensor_mul(
                                expS_t[:, q0 : q0 + P],
                                expS_t[:, q0 : q0 + P],
                                tri_sb[:],
                            )
                            jb_hooks(jb)
                            if pending is not None:
                                emit_attnv(*pending)
                            pending = (jb, expS_t, 0)
                        # jbs 12..15 are narrow: pack pairs into one psum tile
                        # (bank-aligned) and drain each pair with ONE exp
                        for ja in (12, 14):
                            pss = ps_s.tile([P, 1024], F32, tag="pss", name="pss")
                            expS2 = spool.tile(
                                [P, 1024], BF16, tag="expSL", name="eSL"
                            )
                            locs = {ja: 0, ja + 1: 512}
                            span = 512 + (T - P * (ja + 1))
                            for j in (ja, ja + 1):
                                q0j = P * j
                                nc.tensor.matmul(
                                    pss[:, locs[j] : locs[j] + T - q0j],
                                    kTz[:, pr, ts(j, P)],
                                    qT2[:, pr, q0j:T],
                                    start=True,
                                    stop=True,
                                )
                            nc.scalar.activation(
                                expS2[:, 0:span],
                                pss[:, 0:span],
                                EXP,
                                scale=float(SCALE / (W8SCALE * W8SCALE)),
                            )
                            for j in (ja, ja + 1):
                                nc.vector.tensor_mul(
                                    expS2[:, locs[j] : locs[j] + P],
                                    expS2[:, locs[j] : locs[j] + P],
                                    tri_sb[:],
                                )
                                jb_hooks(j)
                                if pending is not None:
                                    emit_attnv(*pending)
                                pending = (j, expS2, P * j - locs[j])
                        emit_attnv(*pending)

                        # evict the last quarter; psoT is then free
                        nc.vector.tensor_copy(
                            aU[base : base + DK, pr, 1536:2048],
                            psoT1[0:DK, 512:1024],
                        )
                        nc.vector.tensor_copy(
                            den1[:, 1536:2048], psoT1[DK : DK + 1, 512:1024]
                        )
                        if h == HPC - 2:
                            deferred_norm[0], deferred_mul[0] = emit_norm_chain(
                                h, pr, base, den1, 1024, 2048
                            )
                        elif h == HPC - 1:
                            t, m = emit_norm_chain(h, pr, base, den1, 1024, 2048)
                            t()
                            m()
                        else:
                            deferred_norm[0], deferred_mul[0] = emit_norm_chain(
                                h, pr, base, den1, 0, 2048
                            )
                    for dfr in (deferred_norm, deferred_mul, self_mul):
                        if dfr[0] is not None:
                            dfr[0]()
                            dfr[0] = None

            # ---- phase 3: output projection (partial, transposed) ----
            with (
                tc.tile_pool(name="outp", bufs=6) as ypool,
                tc.tile_pool(name="ps_y", bufs=2, space="PSUM") as ps_y,
            ):
                yTr = yT.rearrange("(ob p) t -> p ob t", p=P)
                for qc4 in range(4):
                    for oblk in range(KB):
                        psy = ps_y.tile(
                            [P, 512], F32, tag=f"psy{oblk % 4}", name="psy"
                        )
                        for cb in range(OB):
                            nc.tensor.matmul(
                                psy[:],
                                wo_sb[:, cb, ts(oblk, P)],
                                aU[:, cb, ts(qc4, 512)],
                                start=(cb == 0),
                                stop=(cb == OB - 1),
                            )
                        ysb = ypool.tile([P, 512], BF16, tag="ysb", name="ysb")
                        if oblk % 2 == 0:
                            nc.vector.tensor_copy(ysb[:], psy[:])
                        else:
                            nc.scalar.copy(ysb[:], psy[:])
                        nc.sync.dma_start(
                            yTr[:, oblk, ts(qc4, 512)], ysb[:]
                        )

    nc.compile()
    return nc


def _host_inputs(x, mask, Wq, bq_v, Wk, bk_v, Wv, bv_v, Wo, bo_v):
    f32 = np.float32
    bf16 = ml_dtypes.bfloat16
    f8 = ml_dtypes.float8_e4m3
    wqT = np.ascontiguousarray(np.asarray(Wq, f32).T) * W8SCALE
    wkT = np.ascontiguousarray(np.asarray(Wk, f32).T) * W8SCALE
    wvT = np.ascontiguousarray(np.asarray(Wv, f32).T).astype(bf16)
    woT = np.ascontiguousarray(np.asarray(Wo, f32).T).astype(bf16)
    # exact v/o bias fold: softmax rows sum to 1, so v+bv adds bv to attn out
    bo_eff = np.asarray(bo_v, f32) + np.asarray(bv_v, f32) @ np.asarray(Wo, f32).T

    # tri[k, q] = 1 where k <= q (causal keep within the diagonal block)
    kk = np.arange(P)
    tri = (kk[:, None] <= kk[None, :]).astype(f32).astype(bf16)

    bqa = np.asarray(bq_v, f32)
    bka = np.asarray(bk_v, f32)
    zebA = np.zeros((P, 1), f32)
    zebA[0:DK] = 1.0

    halves = []
    for half in range(2):
        sl = slice(half * CPC, (half + 1) * CPC)
        # biases carry the fp8 weight pre-scale; exp scale divides it out
        bq_p = np.ascontiguousarray(bqa[sl].reshape(OB, P).T) * W8SCALE
        bk_p = np.ascontiguousarray(bka[sl].reshape(OB, P).T) * W8SCALE
        halves.append(
            {
                "wq8": np.ascontiguousarray(wqT[:, sl]).astype(f8),
                "wk8": np.ascontiguousarray(wkT[:, sl]).astype(f8),
                "wv": np.ascontiguousarray(wvT[:, sl]),
                "wo": np.ascontiguousarray(woT[sl, :]),
                "bq": bq_p,
                "bkzA": bk_p * zebA,
                "bkzB": bk_p * (1.0 - zebA),
                "tri": tri,
            }
        )

    xn = np.asarray(x, f32)
    in_maps = []
    for core in range(8):
        b, half = divmod(core, 2)
        m = dict(halves[half])
        xTb = np.ascontiguousarray(xn[b].T)
        m["xT"] = xTb.astype(bf16)
        m["x8"] = xTb.astype(f8)
        in_maps.append(m)
    return in_maps, bo_eff


def _run(inputs, trace=False):
    if "nc" not in _cache:
        _cache["nc"] = _build()
    nc = _cache["nc"]
    in_maps, bo_eff = _host_inputs(
        inputs["x"], inputs["mask"],
        inputs["Wq"], inputs["bq"], inputs["Wk"], inputs["bk"],
        inputs["Wv"], inputs["bv"], inputs["Wo"], inputs["bo"],
    )
    res = run_bass_kernel_spmd(nc, in_maps, list(range(8)), trace=trace)
    out = np.empty((B, T, C), np.float32)
    for b in range(B):
        yA = res.results[2 * b]["yT"].astype(np.float32)
        yB = res.results[2 * b + 1]["yT"].astype(np.float32)
        out[b] = (yA + yB).T + bo_eff
    return out, res


def kernel(**inputs):
    out, _ = _run(inputs, trace=False)
    return out
